# revision 36
# baseline (speedup 1.0000x reference)
"""AdaPeakConv2D Trainium2 kernel — 8-core data parallel over batch.

Self-contained: hardcodes shapes B=8, C=32, H=W=128, OUT=32, KH=KW=4.

Decomposition (validated against the reference in numpy):
  out = center(1x1 conv) + b_pk
        - sum_d [ cP_d * F_d(P-shift) + cQ_d * F_d(Q-shift) ]
        - rb corner strips (w=127 bottom / h=127 left)
  - BandEst preact via one K=96 matmul (bf16 hi/lo split for ~fp32
    accuracy) + zero-pad shifted adds; BN stats via a 16-float AllReduce.
  - Directional 4-tap convs as two K=128 tap-packed matmuls (M=64) over
    shifted bf16 copies of the edge-padded image (XP4_H / XP4_V).
  - 2-way integer-shift select folded into broadcast coefficient planes.
  - Final combine: center matmul + two K=128 "sum" matmuls in PSUM.
"""
import os
import numpy as np
import ml_dtypes

import concourse.bass as bass
import concourse.bass_isa as bass_isa
import concourse.bacc as bacc
import concourse.tile as tile
import concourse.mybir as mybir
from concourse.bass import MemorySpace
from concourse.bass_utils import run_bass_kernel_spmd

BF16 = mybir.dt.bfloat16
F32 = mybir.dt.float32
AX = mybir.AxisListType.X
ALU = mybir.AluOpType
ACT = mybir.ActivationFunctionType

B, C, H, W = 8, 32, 128, 128
OUT = 32
HW = H * W          # 16384
HP, WP = 132, 132   # edge pad 2
N_CORES = 8
INV_N = 1.0 / (B * H * W)
EPS = 1e-5

NCHUNK_SEL = 8          # select-stage chunks (16 image rows each)
SEL_ROWS = H // NCHUNK_SEL


def _bf(x):
    return np.asarray(x, dtype=ml_dtypes.bfloat16)


def host_prep(inputs):
    """Pack weights for the device. Returns dict of np arrays (per-core
    replicated) keyed by dram tensor name."""
    wpk = np.asarray(inputs['w_pk'], np.float32)
    Wm = wpk.reshape(OUT, C, 16)

    # H-group lhsT [128=(g,c), 64]: cols 0:32 top (n=g), 32:64 bot (n=8+g)
    W_H = np.zeros((128, 64), np.float32)
    # V-group: right (n=4+g), left (n=12+g)
    W_V = np.zeros((128, 64), np.float32)
    for g in range(4):
        for c in range(C):
            W_H[g * 32 + c, 0:32] = Wm[:, c, g]
            W_H[g * 32 + c, 32:64] = Wm[:, c, 8 + g]
            W_V[g * 32 + c, 0:32] = Wm[:, c, 12 + g]   # left
            W_V[g * 32 + c, 32:64] = Wm[:, c, 4 + g]   # right

    # sum matmul: -identity blocks [128, 32]
    W_S = np.zeros((128, 32), np.float32)
    for blk in range(4):
        for o in range(32):
            W_S[blk * 32 + o, o] = -1.0

    # BandEst [96, 28]; col order ch*7+j, ch = (v0, hz0, v1, hz1)
    wv = np.asarray(inputs['w_vrt'], np.float32)    # (2,C,7,1)
    wh = np.asarray(inputs['w_hrz'], np.float32)    # (2,C,1,7)
    WB_full = np.zeros((C, 28), np.float32)
    for j in range(7):
        WB_full[:, 0 * 7 + j] = wv[0, :, j, 0]
        WB_full[:, 1 * 7 + j] = wh[0, :, 0, j]
        WB_full[:, 2 * 7 + j] = wv[1, :, j, 0]
        WB_full[:, 3 * 7 + j] = wh[1, :, 0, j]
    WB_hi = _bf(WB_full).astype(np.float32)
    WB_lo = WB_full - WB_hi
    # stack blocks pair with bs = [x_hi; x_hi; x_lo]:
    #   Whi@x_hi + Wlo@x_hi + Whi@x_lo  ~=  W@x to ~2^-17
    W_B = np.concatenate([WB_hi, WB_lo, WB_hi], axis=0)   # [96, 28]
    # center 1x1 conv folded into the same matmul: cols 28:60 = W_C acting
    # on x_hi (rows 0:32) + x_lo (rows 64:96)  ->  exact-x center plane
    Wc = Wm.sum(-1)                              # [o, c]
    W_B2 = np.zeros((96, 60), np.float32)
    W_B2[:, 0:28] = W_B
    for c in range(C):
        W_B2[c, 28:60] = Wc[:, c]
        W_B2[64 + c, 28:60] = Wc[:, c]
    bias60 = np.zeros((60, 1), np.float32)
    bias60[28:60, 0] = np.asarray(inputs['b_pk'], np.float32)

    W_G11 = Wm[:, :, 11].T.copy()               # [c, o]
    W_G15 = Wm[:, :, 15].T.copy()

    # broadcast lhsT: B4[d, 32d:32d+32] = 1 -> psum[128] = 4 dir blocks
    B4 = np.zeros((4, 128), np.float32)
    for d in range(4):
        B4[d, 32 * d:32 * d + 32] = 1.0

    gv = np.asarray(inputs['g_vrt'], np.float32)
    bev = np.asarray(inputs['be_vrt'], np.float32)
    gh = np.asarray(inputs['g_hrz'], np.float32)
    beh = np.asarray(inputs['be_hrz'], np.float32)
    # ch order (v0, hz0, v1, hz1)
    gamma = np.array([gv[0], gh[0], gv[1], gh[1]], np.float32)
    beta = np.array([bev[0], beh[0], bev[1], beh[1]], np.float32)
    bnp = np.concatenate([gamma, beta]).reshape(1, 8)

    bpk = np.asarray(inputs['b_pk'], np.float32).reshape(32, 1)

    return {
        'W_H': _bf(W_H), 'W_V': _bf(W_V), 'W_S': _bf(W_S),
        'W_B': _bf(W_B2), 'W_G11': _bf(W_G11), 'W_G15': _bf(W_G15),
        'bnp': bnp, 'bpk': bpk, 'B4': _bf(B4), 'bias60': bias60,
    }


def build(nc):
    x_d = nc.dram_tensor("x", [C, H, W], F32, kind="ExternalInput")
    wh_d = nc.dram_tensor("W_H", [128, 64], BF16, kind="ExternalInput")
    wv_d = nc.dram_tensor("W_V", [128, 64], BF16, kind="ExternalInput")
    ws_d = nc.dram_tensor("W_S", [128, 32], BF16, kind="ExternalInput")
    wb_d = nc.dram_tensor("W_B", [96, 60], BF16, kind="ExternalInput")
    wg11_d = nc.dram_tensor("W_G11", [32, 32], BF16, kind="ExternalInput")
    wg15_d = nc.dram_tensor("W_G15", [32, 32], BF16, kind="ExternalInput")
    bnp_d = nc.dram_tensor("bnp", [1, 8], F32, kind="ExternalInput")
    bpk_d = nc.dram_tensor("bpk", [32, 1], F32, kind="ExternalInput")
    b4_d = nc.dram_tensor("B4", [4, 128], BF16, kind="ExternalInput")
    bias60_d = nc.dram_tensor("bias60", [60, 1], F32, kind="ExternalInput")
    out_d = nc.dram_tensor("out", [OUT, H, W], F32, kind="ExternalOutput")

    with tile.TileContext(nc) as tc:
        _graph(nc, tc, x_d, wh_d, wv_d, ws_d, wb_d, wg11_d, wg15_d,
               bnp_d, bpk_d, b4_d, bias60_d, out_d)
    return nc


def _graph(nc, tc, x_d, wh_d, wv_d, ws_d, wb_d, wg11_d, wg15_d,
           bnp_d, bpk_d, b4_d, bias60_d, out_d):
    main = tc.alloc_tile_pool(name="main", bufs=1)
    dram = tc.alloc_tile_pool(name="dram", bufs=1, space="DRAM")
    late = tc.alloc_tile_pool(name="late", bufs=1)
    xp4hp = tc.alloc_tile_pool(name="xp4hp", bufs=1)
    xp4vp = tc.alloc_tile_pool(name="xp4vp", bufs=1)
    psum_acc = tc.alloc_tile_pool(name="psum_acc", bufs=2, space="PSUM")
    early = tc.alloc_tile_pool(name="early", bufs=1)
    psum_be = tc.alloc_tile_pool(name="psum_be", bufs=2, space="PSUM")
    chunkE = tc.alloc_tile_pool(name="chunkE", bufs=2)

    x_flat = x_d[:].rearrange("c h w -> c (h w)")

    # ---------------- weights ----------------
    w_h = main.tile([128, 64], BF16); nc.sync.dma_start(w_h[:], wh_d[:])
    w_v = main.tile([128, 64], BF16); nc.sync.dma_start(w_v[:], wv_d[:])
    w_s = main.tile([128, 32], BF16); nc.sync.dma_start(w_s[:], ws_d[:])
    w_b = main.tile([96, 60], BF16); nc.sync.dma_start(w_b[:], wb_d[:])
    w_g11 = main.tile([32, 32], BF16); nc.sync.dma_start(w_g11[:], wg11_d[:])
    w_g15 = main.tile([32, 32], BF16); nc.sync.dma_start(w_g15[:], wg15_d[:])
    bnp = main.tile([1, 8], F32); nc.sync.dma_start(bnp[:], bnp_d[:])
    bpk = main.tile([32, 1], F32); nc.sync.dma_start(bpk[:], bpk_d[:])
    b4 = main.tile([4, 128], BF16); nc.sync.dma_start(b4[:], b4_d[:])
    bias60 = main.tile([60, 1], F32); nc.sync.dma_start(bias60[:], bias60_d[:])

    # -------- BandEst stack + matmul + xpad interior, chunked --------
    xpad = early.tile([C, HP, WP], BF16)
    y_dram = dram.tile([60, HW], F32)
    NB = 8
    bchunk = HW // NB              # 2048 = 16 image rows
    for k in range(NB):
        sl = slice(k * bchunk, (k + 1) * bchunk)
        xs_c = chunkE.tile([C, bchunk], F32, tag="xs_c")
        nc.sync.dma_start(xs_c[:], x_flat[:, sl])
        bs_c = chunkE.tile([96, bchunk], BF16, tag="bs_c")
        nc.gpsimd.dma_start(bs_c[0:32, :], x_flat[:, sl])    # cast f32->bf16
        nc.gpsimd.dma_start(bs_c[32:64, :], x_flat[:, sl])
        subq = nc.vector if k % 2 == 0 else nc.gpsimd
        subq.tensor_tensor(bs_c[64:96, :], xs_c[:], bs_c[0:32, :],
                           ALU.subtract)
        # xpad interior rows for this chunk (sbuf->sbuf DMA, strided out)
        nc.sync.dma_start(
            xpad[:, 16 * k + 2:16 * k + 18, 2:130],
            bs_c[0:32, :].rearrange("c (r w) -> c r w", w=W))
        for i in range(bchunk // 1024):
            pb = psum_be.tile([60, 2, 512], F32, tag="pbe")
            for u in range(2):
                o0 = i * 1024 + u * 512
                nc.tensor.matmul(pb[:, u, :], w_b[:], bs_c[:, o0:o0 + 512],
                                 start=True, stop=True)
            yc = chunkE.tile([60, 1024], F32, tag="yc")
            nc.scalar.activation(yc[:], pb[:].rearrange("p a b -> p (a b)"),
                                 ACT.Identity, bias=bias60[:, 0:1])
            yq = nc.gpsimd if i == 0 else nc.sync
            o0 = k * bchunk + i * 1024
            yq.dma_start(y_dram[0:60, o0:o0 + 1024], yc[:])
    psum_be.release()
    chunkE.release()

    # plane transpose: y_dram rows -> [h=partition, m, w] planes; v-channel
    # row shifts baked into the DMAs, edges zeroed by one memset.
    ytt = early.tile([128, 28, W], F32)
    nc.gpsimd.memset(ytt[:], 0.0)
    _yq = [nc.scalar, nc.gpsimd]
    for ch in (0, 2):
        for j in range(7):
            m = ch * 7 + j
            s = j - 3
            lo, hi = max(0, -s), min(128, 128 - s)
            _yq[(ch + j) % 2].dma_start(
                ytt[lo:hi, m, :],
                y_dram[m:m + 1, (lo + s) * W:(hi + s) * W])
    nc.scalar.dma_start(ytt[:, 7:14, :],
                        y_dram[7:14, :].rearrange("m (h w) -> h m w", w=W))
    nc.gpsimd.dma_start(ytt[:, 21:28, :],
                        y_dram[21:28, :].rearrange("m (h w) -> h m w", w=W))

    # preact planes stacked [128, 4, 128]; ch order (v0=top, hz0=left,
    # v1=bot, hz1=right) -> dir stack (top, left, bot, right)
    preS = main.tile([128, 4, W], F32)
    for ch in range(4):
        nc.vector.tensor_copy(preS[:, ch, :], ytt[:, ch * 7 + 3, :])
        for j in [0, 1, 2, 4, 5, 6]:
            s = j - 3
            if ch in (0, 2):   # rows pre-shifted in the DMA: full add
                nc.vector.tensor_tensor(preS[:, ch, :], preS[:, ch, :],
                                        ytt[:, ch * 7 + j, :], ALU.add)
            else:              # horizontal: col (free) shifts
                lo, hi = max(0, -s), min(128, 128 - s)
                nc.vector.tensor_tensor(preS[:, ch, lo:hi], preS[:, ch, lo:hi],
                                        ytt[:, ch * 7 + j, lo + s:hi + s],
                                        ALU.add)

    # ---- BN stats early so the AllReduce overlaps the conv phase ----
    colsum = main.tile([128, 8], F32)
    sqt = main.tile([H, W], F32)
    for ch in range(4):
        nc.vector.tensor_reduce(colsum[:, ch:ch + 1], preS[:, ch, :], AX, ALU.add)
        nc.vector.tensor_tensor(sqt[:], preS[:, ch, :], preS[:, ch, :], ALU.mult)
        nc.vector.tensor_reduce(colsum[:, 4 + ch:5 + ch], sqt[:], AX, ALU.add)
    sums = main.tile([128, 8], F32)
    nc.gpsimd.partition_all_reduce(sums[:], colsum[:], 128,
                                   bass_isa.ReduceOp.add)

    # ---------------- padded image edges + XP4 (via DRAM) ----------------
    for dst, src_ in [(0, 2), (1, 2), (130, 129), (131, 129)]:
        nc.vector.tensor_copy(xpad[:, 2:130, dst:dst + 1],
                              xpad[:, 2:130, src_:src_ + 1])
    for dst, src_ in [(0, 2), (1, 2), (130, 129), (131, 129)]:
        nc.vector.tensor_copy(xpad[:, dst, :], xpad[:, src_, :])

    xpad_dram = dram.tile([C, HP * WP], BF16)
    nc.sync.dma_start(xpad_dram[:], xpad[:].rearrange("c a b -> c (a b)"))
    xp4h = xp4hp.tile([128, HP * WP], BF16)
    xp4v = xp4vp.tile([128, HP * WP], BF16)
    for g in range(4):
        nc.sync.dma_start(xp4h[32 * g:32 * g + 32, 0:HP * WP - g],
                          xpad_dram[:, g:HP * WP])
        nc.sync.dma_start(xp4v[32 * g:32 * g + 32, 0:HP * WP - g * WP],
                          xpad_dram[:, g * WP:HP * WP])
    vh = xp4h[:].rearrange("p (r c) -> p r c", c=WP)   # [128, 132, 132]
    vv = xp4v[:].rearrange("p (r c) -> p r c", c=WP)

    ar_in = dram.tile([1, 8], F32)
    ar_out = dram.tile([1, 8], F32)
    nc.gpsimd.dma_start(ar_in[:], sums[0:1, :])
    nc.gpsimd.collective_compute(
        "AllReduce", ALU.add,
        replica_groups=[list(range(N_CORES))],
        ins=[ar_in.opt()], outs=[ar_out.opt()],
    )
    gs = main.tile([1, 8], F32)
    nc.gpsimd.dma_start(gs[:], ar_out[:])

    early.release()
    psum_hv = tc.alloc_tile_pool(name="psum_hv", bufs=1, space="PSUM")

    # ---------------- directional convs -> F planes (bf16) ----------------
    # fhv 0:64 = F_H [64, 129, 128]: top(0:32) rr=F_top[max(rr-2,0)];
    #   bot(32:64) rr=F_bot[min(rr+4,131)], psum col w+1.
    #   select reads: P = [:, h+1, w], Q = [:, h, w]
    # fhv 64:128 = F_V [64, 128, 129]: left(0:32) cc=F_left[r+1,max(cc-2,0)];
    #   right(32:64) cc=F_right[r, min(cc+4,131)]
    #   select reads: P = [:, h, w+1], Q = [:, h, w]
    fhv = late.tile([128, 129 * 128], BF16)
    f_h = fhv[0:64].rearrange("p (r c) -> p r c", r=129)
    f_v = fhv[64:128].rearrange("p (r c) -> p r c", r=128)

    _ev_state = [0]
    def _evict(dst, srcv):
        e = _ev_state[0] % 2
        _ev_state[0] += 1
        if e == 0:
            nc.scalar.activation(dst, srcv, ACT.Identity)
        else:
            nc.vector.tensor_copy(dst, srcv)

    # interleave H and V 6-row chunks; psum [64, 2, 512] (bank-aligned halves)
    for k in range(22):
        r0 = 6 * k
        ph = psum_hv.tile([64, 2, 512], F32, tag="ph")
        for u in range(2):
            nc.tensor.matmul(ph[:, u, 0:387], w_h[:],
                             vh[:, r0 + 3 * u:r0 + 3 * u + 3, 0:129],
                             start=True, stop=True)
        phv = ph[:, :, 0:387].rearrange("p a (r c) -> p a r c", c=129)
        # top: rr = r+2, r in [0..126]
        lo, hi = r0, min(r0 + 6, 127)
        if lo < hi:
            for hf, a, b, oo in _psum_pieces(lo - r0, hi - r0):
                _evict(f_h[0:32, lo + oo + 2:lo + oo + 2 + (b - a), :],
                       phv[0:32, hf, a:b, 0:128])
        # bot: rr = r-4, r in [4..131]
        lo, hi = max(r0, 4), min(r0 + 6, 132)
        if lo < hi:
            for hf, a, b, oo in _psum_pieces(lo - r0, hi - r0):
                _evict(f_h[32:64, lo + oo - 4:lo + oo - 4 + (b - a), :],
                       phv[32:64, hf, a:b, 1:129])

        pvt = psum_hv.tile([64, 2, 512], F32, tag="pv")
        hp0 = 6 * k
        nrows = min(6, 129 - hp0)
        if nrows <= 0:
            continue
        for u in range(2):
            nr = min(3, 129 - hp0 - 3 * u)
            if nr > 0:
                nc.tensor.matmul(pvt[:, u, 0:nr * 132], w_v[:],
                                 vv[:, hp0 + 3 * u:hp0 + 3 * u + nr, 0:132],
                                 start=True, stop=True)
        pvv = pvt[:, :, 0:396].rearrange("p a (r c) -> p a r c", c=132)
        # left (0:32): r = hp-1, hp in [1..128]; cc = col+2 for cols 0..126
        lo, hi = max(hp0, 1), min(hp0 + nrows, 129)
        if lo < hi:
            for hf, a, b, oo in _psum_pieces(lo - hp0, hi - hp0):
                _evict(f_v[0:32, lo + oo - 1:lo + oo - 1 + (b - a), 2:129],
                       pvv[0:32, hf, a:b, 0:127])
        # right (32:64): r = hp in [0..127]; cc <- psum col cc+4
        lo, hi = hp0, min(hp0 + nrows, 128)
        if lo < hi:
            for hf, a, b, oo in _psum_pieces(lo - hp0, hi - hp0):
                _evict(f_v[32:64, lo + oo:lo + oo + (b - a), 0:128],
                       pvv[32:64, hf, a:b, 4:132])

    # one-time dup rows/cols (self copies after main fills)
    nc.vector.tensor_copy(f_h[0:32, 0, :], f_h[0:32, 2, :])
    nc.vector.tensor_copy(f_h[0:32, 1, :], f_h[0:32, 2, :])
    nc.vector.tensor_copy(f_h[32:64, 128, :], f_h[32:64, 127, :])
    nc.vector.tensor_copy(f_v[0:32, :, 0:1], f_v[0:32, :, 2:3])
    nc.vector.tensor_copy(f_v[0:32, :, 1:2], f_v[0:32, :, 2:3])
    nc.vector.tensor_copy(f_v[32:64, :, 128:129], f_v[32:64, :, 127:128])

    psum_hv.release()
    psum_bc = tc.alloc_tile_pool(name="psum_bc", bufs=2, space="PSUM")
    xp4vp.release()

    # ---------------- rb corner strip matmuls ----------------
    g11rhs = main.tile([32, 132], BF16)
    nc.sync.dma_start(g11rhs[:], vh[0:32, :, 131])
    pg = psum_acc.tile([32, 512], F32, tag="acc")
    nc.tensor.matmul(pg[0:32, 0:132], w_g11[:], g11rhs[:],
                     start=True, stop=True)
    g11e = main.tile([32, 134], F32)
    nc.scalar.activation(g11e[:, 0:132], pg[0:32, 0:132], ACT.Identity)
    nc.scalar.activation(g11e[:, 132:133], pg[0:32, 131:132], ACT.Identity)
    nc.scalar.activation(g11e[:, 133:134], pg[0:32, 131:132], ACT.Identity)
    pg2 = psum_acc.tile([32, 512], F32, tag="acc")
    nc.tensor.matmul(pg2[0:32, 0:132], w_g15[:], vh[0:32, 131, 0:132],
                     start=True, stop=True)
    g15e = main.tile([32, 133], F32)
    nc.scalar.activation(g15e[:, 1:133], pg2[0:32, 0:132], ACT.Identity)
    nc.scalar.activation(g15e[:, 0:1], pg2[0:32, 0:1], ACT.Identity)
    xp4hp.release()

    # ---------------- BN consts from the early AllReduce ----------------
    mu = main.tile([1, 4], F32)
    nc.vector.tensor_scalar(mu[:], gs[:, 0:4], INV_N, None, ALU.mult)
    ex2 = main.tile([1, 4], F32)
    nc.vector.tensor_scalar(ex2[:], gs[:, 4:8], INV_N, None, ALU.mult)
    var = main.tile([1, 4], F32)
    nc.vector.tensor_tensor(var[:], mu[:], mu[:], ALU.mult)
    nc.vector.tensor_tensor(var[:], ex2[:], var[:], ALU.subtract)
    nc.vector.tensor_scalar(var[:], var[:], EPS, None, ALU.add)
    sd = main.tile([1, 4], F32)
    nc.scalar.sqrt(sd[:], var[:])
    rsq = main.tile([1, 4], F32)
    nc.vector.reciprocal(rsq[:], sd[:])
    zscale = main.tile([1, 4], F32)
    nc.vector.tensor_tensor(zscale[:], rsq[:], bnp[:, 0:4], ALU.mult)
    zbias = main.tile([1, 4], F32)
    nc.vector.tensor_tensor(zbias[:], mu[:], zscale[:], ALU.mult)
    nc.vector.tensor_tensor(zbias[:], bnp[:, 4:8], zbias[:], ALU.subtract)
    cons = main.tile([128, 8], F32)   # bcast: zscale 0:4, zbias 4:8
    nc.gpsimd.partition_broadcast(cons[:, 0:4], zscale[:])
    nc.gpsimd.partition_broadcast(cons[:, 4:8], zbias[:])

    # ------------- coefficient planes, dir-stacked [128, 4, 128] ----------
    # dir stack order: (top, left, bot, right); minus dirs = 0:2, plus = 2:4
    coeffp = tc.alloc_tile_pool(name="coeffp", bufs=1)
    ih = main.tile([H, W], mybir.dt.int32)
    nc.gpsimd.iota(ih[:], pattern=[[0, W]], base=0, channel_multiplier=1)
    iw = main.tile([H, W], mybir.dt.int32)
    nc.gpsimd.iota(iw[:], pattern=[[1, W]], base=0, channel_multiplier=0)
    idxb1 = coeffp.tile([128, 4, W], F32)   # IDX - 1 (minus) / IDX + 4 (plus)
    idxb2 = coeffp.tile([128, 4, W], F32)   # IDX (minus) / IDX + 4 (plus)
    for col, (srci, o1, o2) in enumerate([(0, -1.0, 0.0), (1, -1.0, 0.0),
                                          (0, 4.0, 4.0), (1, 4.0, 4.0)]):
        srct = ih if srci == 0 else iw
        nc.vector.tensor_scalar(idxb1[:, col, :], srct[:], o1, None, ALU.add)
        nc.vector.tensor_scalar(idxb2[:, col, :], srct[:], o2, None, ALU.add)

    z = coeffp.tile([128, 4, W], F32)
    for ch in range(4):
        nc.vector.tensor_scalar(z[:, ch, :], preS[:, ch, :],
                                cons[:, ch:ch + 1], cons[:, 4 + ch:5 + ch],
                                ALU.mult, ALU.add)
    gb = coeffp.tile([128, 4, W], F32)
    nc.scalar.activation(gb[:].rearrange("p a b -> p (a b)"),
                         z[:].rearrange("p a b -> p (a b)"), ACT.Sigmoid)
    nc.vector.tensor_scalar(gb[:], gb[:], 2.0, None, ALU.mult)
    m2 = coeffp.tile([128, 4, W], F32)
    nc.vector.tensor_scalar(m2[:], z[:], 0.0, None, ALU.is_gt)

    q = coeffp.tile([128, 4, W], F32)
    nc.vector.tensor_tensor(q[:, 0:2, :], idxb1[:, 0:2, :], m2[:, 0:2, :],
                            ALU.subtract)
    nc.vector.tensor_tensor(q[:, 2:4, :], idxb1[:, 2:4, :], m2[:, 2:4, :],
                            ALU.add)
    nc.vector.tensor_scalar(q[:, 0:2, :], q[:, 0:2, :], 0.0, None, ALU.max)
    nc.vector.tensor_scalar(q[:, 2:4, :], q[:, 2:4, :], 131.0, None, ALU.min)
    pcl = coeffp.tile([128, 4, W], F32)
    nc.vector.tensor_tensor(pcl[:, 0:2, :], idxb2[:, 0:2, :], gb[:, 0:2, :],
                            ALU.subtract)
    nc.vector.tensor_tensor(pcl[:, 2:4, :], idxb2[:, 2:4, :], gb[:, 2:4, :],
                            ALU.add)
    nc.vector.tensor_scalar(pcl[:, 0:2, :], pcl[:, 0:2, :], 0.0, None, ALU.max)
    nc.vector.tensor_scalar(pcl[:, 2:4, :], pcl[:, 2:4, :], 131.0, None,
                            ALU.min)
    wlt = coeffp.tile([128, 4, W], F32)
    nc.vector.tensor_tensor(wlt[:], q[:], pcl[:], ALU.subtract)
    nc.vector.tensor_scalar(wlt[:], wlt[:], 1.0, None, ALU.add)
    # cq = wlt * [m2, m2, 1-m2, 1-m2]; cp = wlt - cq
    bmul = coeffp.tile([128, 4, W], F32)
    nc.vector.tensor_tensor(bmul[:], wlt[:], m2[:], ALU.mult)
    cqS = coeffp.tile([128, 4, W], BF16)
    nc.vector.tensor_copy(cqS[:, 0:2, :], bmul[:, 0:2, :])
    nc.vector.tensor_tensor(cqS[:, 2:4, :], wlt[:, 2:4, :], bmul[:, 2:4, :],
                            ALU.subtract)
    cpS = coeffp.tile([128, 4, W], BF16)
    nc.vector.tensor_tensor(cpS[:], wlt[:], cqS[:], ALU.subtract)

    # ab_dram rows (order = S partition groups): cp: [top, bot, left, right]
    # cq rows 4:8 same order. Stack cols (t,l,b,r) -> rows via 2 DMAs each.
    ab_dram = dram.tile([8, HW], BF16)
    for base, tsrc in ((0, cpS), (4, cqS)):
        nc.sync.dma_start(
            ab_dram[base:base + 2, :].rearrange("d (h w) -> h d w", w=W),
            tsrc[:, 0:4:2, :])          # cols (top, bot) -> rows +0,+1
        nc.gpsimd.dma_start(
            ab_dram[base + 2:base + 4, :].rearrange("d (h w) -> h d w", w=W),
            tsrc[:, 1:4:2, :])          # cols (left, right) -> rows +2,+3


    # ---------------- rb corner strip coefficients ----------------
    # bottom strip at (h, 127): dir col 2; [128,1] partition-major
    ihf1 = main.tile([128, 1], F32)
    nc.vector.tensor_copy(ihf1[:], ih[:, 0:1])
    m2c = main.tile([128, 1], F32)
    nc.vector.tensor_copy(m2c[:], m2[:, 2, 127:128])
    gbc = main.tile([128, 1], F32)
    nc.vector.tensor_copy(gbc[:], gb[:, 2, 127:128])
    qs = main.tile([128, 1], F32)
    nc.vector.tensor_tensor(qs[:], ihf1[:], m2c[:], ALU.add)
    nc.vector.tensor_scalar(qs[:], qs[:], 5.0, 131.0, ALU.add, ALU.min)
    ps_ = main.tile([128, 1], F32)
    nc.vector.tensor_tensor(ps_[:], ihf1[:], gbc[:], ALU.add)
    nc.vector.tensor_scalar(ps_[:], ps_[:], 4.0, 131.0, ALU.add, ALU.min)
    wrb = main.tile([128, 1], F32)
    nc.vector.tensor_tensor(wrb[:], qs[:], ps_[:], ALU.subtract)
    nc.vector.tensor_scalar(wrb[:], wrb[:], 1.0, None, ALU.subtract)
    cbs = main.tile([128, 1], F32)
    nc.vector.tensor_tensor(cbs[:], wrb[:], m2c[:], ALU.mult)
    cas = main.tile([128, 1], F32)
    nc.vector.tensor_tensor(cas[:], wrb[:], cbs[:], ALU.subtract)
    strip_dram = dram.tile([4, 128], F32)
    nc.sync.dma_start(strip_dram[0:1, :], cas[:])
    nc.sync.dma_start(strip_dram[1:2, :], cbs[:])

    # left strip at (127, w): dir col 1; [1, 128] via DMA off partition 127
    m2r = main.tile([1, 128], F32)
    nc.sync.dma_start(m2r[:], m2[127:128, 1, :])
    gbr = main.tile([1, 128], F32)
    nc.sync.dma_start(gbr[:], gb[127:128, 1, :])
    iwf1 = main.tile([1, 128], F32)
    nc.vector.tensor_copy(iwf1[:], iw[0:1, :])
    qs2 = main.tile([1, 128], F32)
    nc.vector.tensor_tensor(qs2[:], iwf1[:], m2r[:], ALU.subtract)
    nc.vector.tensor_scalar(qs2[:], qs2[:], 0.0, None, ALU.max)
    ps2 = main.tile([1, 128], F32)
    nc.vector.tensor_tensor(ps2[:], iwf1[:], gbr[:], ALU.subtract)
    nc.vector.tensor_scalar(ps2[:], ps2[:], 0.0, None, ALU.max)
    wrb2 = main.tile([1, 128], F32)
    nc.vector.tensor_tensor(wrb2[:], qs2[:], ps2[:], ALU.subtract)
    nc.vector.tensor_scalar(wrb2[:], wrb2[:], 1.0, None, ALU.subtract)
    cb2 = main.tile([1, 128], F32)
    nc.vector.tensor_tensor(cb2[:], wrb2[:], m2r[:], ALU.mult)
    ca2 = main.tile([1, 128], F32)
    nc.vector.tensor_tensor(ca2[:], wrb2[:], cb2[:], ALU.subtract)
    nc.sync.dma_start(strip_dram[2:3, :], ca2[:])
    nc.sync.dma_start(strip_dram[3:4, :], cb2[:])

    strips = main.tile([32, 4, 128], F32)
    nc.sync.dma_start(strips[:],
                      strip_dram[:].unsqueeze(0).broadcast_to([32, 4, 128]))
    t1 = main.tile([32, 128], F32)
    nc.vector.tensor_tensor(t1[:], strips[:, 0, :], g11e[:, 5:133], ALU.mult)
    t2 = main.tile([32, 128], F32)
    nc.vector.tensor_tensor(t2[:], strips[:, 1, :], g11e[:, 6:134], ALU.mult)
    nc.vector.tensor_tensor(t1[:], t1[:], t2[:], ALU.add)
    t3 = main.tile([32, 128], F32)
    nc.vector.tensor_tensor(t3[:], strips[:, 2, :], g15e[:, 1:129], ALU.mult)
    t4 = main.tile([32, 128], F32)
    nc.vector.tensor_tensor(t4[:], strips[:, 3, :], g15e[:, 0:128], ALU.mult)
    nc.vector.tensor_tensor(t3[:], t3[:], t4[:], ALU.add)

    # ---------------- select stage + final matmuls ----------------
    # Software-pipelined: stage A(k) (bc matmuls -> bf16 evict -> 2x DVE
    # multiplies) is emitted before stage B(k-1) (acc matmuls + out) so the
    # Tensor stream always has ready work.
    coeffp.release()
    chunkL2 = tc.alloc_tile_pool(name="chunkL2", bufs=3)
    out_flat = out_d[:].rearrange("o h w -> o (h w)")

    def sel_stageA(kc):
        r0 = kc * SEL_ROWS
        pix0 = r0 * W
        npix = SEL_ROWS * W                      # 2048
        s1 = chunkL2.tile([128, SEL_ROWS, W], BF16, tag="s1")   # P-stack
        s2 = chunkL2.tile([128, SEL_ROWS, W], BF16, tag="s2")   # Q-stack
        ab_cp = chunkL2.tile([4, npix], BF16, tag="ab_cp")
        nc.gpsimd.dma_start(ab_cp[:], ab_dram[0:4, pix0:pix0 + npix])
        ab_cq = chunkL2.tile([4, npix], BF16, tag="ab_cq")
        nc.gpsimd.dma_start(ab_cq[:], ab_dram[4:8, pix0:pix0 + npix])
        cen = chunkL2.tile([OUT, npix], F32, tag="cen")
        nc.gpsimd.dma_start(cen[:], y_dram[28:60, pix0:pix0 + npix])
        for hf in range(2):
            rr = r0 + 8 * hf            # image rows rr..rr+8 in this half
            rel = 8 * hf
            cpb = chunkL2.tile([128, 8, W], BF16, tag="cpb")
            cqb = chunkL2.tile([128, 8, W], BF16, tag="cqb")
            for u in range(2):
                o_l = 1024 * hf + 512 * u
                pbc = psum_bc.tile([128, 2, 512], F32, tag="pbc")
                nc.tensor.matmul(pbc[:, 0, :], b4[:],
                                 ab_cp[:, o_l:o_l + 512],
                                 start=True, stop=True)
                nc.tensor.matmul(pbc[:, 1, :], b4[:],
                                 ab_cq[:, o_l:o_l + 512],
                                 start=True, stop=True)
                # evict to bf16 SBUF so the multiplies hit DVE 2x mode
                nc.scalar.activation(
                    cpb[:, 4 * u:4 * u + 4, :].rearrange("p r w -> p (r w)"),
                    pbc[:, 0, :], ACT.Identity)
                nc.scalar.activation(
                    cqb[:, 4 * u:4 * u + 4, :].rearrange("p r w -> p (r w)"),
                    pbc[:, 1, :], ACT.Identity)
            nc.vector.tensor_tensor(s1[0:64, rel:rel + 8, :], cpb[0:64],
                                    f_h[:, rr + 1:rr + 9, :], ALU.mult)
            nc.vector.tensor_tensor(s2[0:64, rel:rel + 8, :], cqb[0:64],
                                    f_h[:, rr:rr + 8, :], ALU.mult)
            nc.vector.tensor_tensor(s1[64:128, rel:rel + 8, :], cpb[64:128],
                                    f_v[:, rr:rr + 8, 1:129], ALU.mult)
            nc.vector.tensor_tensor(s2[64:128, rel:rel + 8, :], cqb[64:128],
                                    f_v[:, rr:rr + 8, 0:128], ALU.mult)
        return s1, s2, cen

    def sel_stageB(kc, s1, s2, cen):
        r0 = kc * SEL_ROWS
        pix0 = r0 * W
        npix = SEL_ROWS * W
        out_c = chunkL2.tile([OUT, SEL_ROWS, W], F32, tag="out_c")
        s1v = s1[:].rearrange("p r w -> p (r w)")
        s2v = s2[:].rearrange("p r w -> p (r w)")
        for i in range(npix // 512):
            acc = psum_acc.tile([32, 512], F32, tag="acc")
            nc.tensor.matmul(acc[:], w_s[:], s1v[:, 512 * i:512 * (i + 1)],
                             start=True, stop=False)
            nc.tensor.matmul(acc[:], w_s[:], s2v[:, 512 * i:512 * (i + 1)],
                             start=False, stop=True)
            nc.vector.tensor_tensor(
                out_c[:, 4 * i:4 * i + 4, :].rearrange("p r w -> p (r w)"),
                acc[:], cen[:, 512 * i:512 * (i + 1)], ALU.add)
        nc.vector.tensor_tensor(out_c[:, :, 127], out_c[:, :, 127],
                                t1[:, r0:r0 + SEL_ROWS], ALU.add)
        if kc == NCHUNK_SEL - 1:
            nc.vector.tensor_tensor(out_c[:, SEL_ROWS - 1, :],
                                    out_c[:, SEL_ROWS - 1, :], t3[:], ALU.add)
        nc.sync.dma_start(out_flat[:, pix0:pix0 + npix],
                          out_c[:].rearrange("o r w -> o (r w)"))

    prev = sel_stageA(0)
    for kc in range(1, NCHUNK_SEL):
        cur = sel_stageA(kc)
        sel_stageB(kc - 1, *prev)
        prev = cur
    sel_stageB(NCHUNK_SEL - 1, *prev)

    chunkL2.release()
    late.release()
    psum_bc.release()
    psum_acc.release()
    dram.release()
    main.release()


def _psum_pieces(rlo, rhi):
    """Split psum row range [rlo, rhi) in [0,6) into per-half pieces.

    Returns list of (half, half_rlo, half_rhi, out_row_offset_from_rlo)."""
    assert 0 <= rlo < rhi <= 6
    pieces = []
    if rlo < 3:
        e = min(rhi, 3)
        pieces.append((0, rlo, e, 0))
    if rhi > 3:
        s = max(rlo, 3)
        pieces.append((1, s - 3, rhi - 3, s - rlo))
    return pieces


_CACHED = {}


def _get_nc():
    if 'nc' not in _CACHED:
        nc = bacc.Bacc(None, target_bir_lowering=False)
        build(nc)
        nc.compile()
        _CACHED['nc'] = nc
    return _CACHED['nc']


def kernel(**inputs):
    nc = _get_nc()
    wd = host_prep(inputs)
    x = np.asarray(inputs['x'], np.float32)
    in_maps = []
    for i in range(N_CORES):
        m = {'x': np.ascontiguousarray(x[i])}
        m.update(wd)
        in_maps.append(m)
    res = run_bass_kernel_spmd(nc, in_maps, core_ids=list(range(N_CORES)))
    outs = [res.results[i]['out'] for i in range(N_CORES)]
    return np.stack(outs, axis=0)


if __name__ == '__main__':
    nc = _get_nc()
    print("build+compile OK")



# revision 37
# speedup vs baseline: 1.0919x; 1.0919x over previous
"""AdaPeakConv2D Trainium2 kernel — 8-core data parallel over batch.

Self-contained: hardcodes shapes B=8, C=32, H=W=128, OUT=32, KH=KW=4.

Decomposition (validated against the reference in numpy):
  out = center(1x1 conv) + b_pk
        - sum_d [ cP_d * F_d(P-shift) + cQ_d * F_d(Q-shift) ]
        - rb corner strips (w=127 bottom / h=127 left)
  - BandEst preact via one K=96 matmul (bf16 hi/lo split for ~fp32
    accuracy) + zero-pad shifted adds; BN stats via a 16-float AllReduce.
  - Directional 4-tap convs as two K=128 tap-packed matmuls (M=64) over
    shifted bf16 copies of the edge-padded image (XP4_H / XP4_V).
  - 2-way integer-shift select folded into broadcast coefficient planes.
  - Final combine: center matmul + two K=128 "sum" matmuls in PSUM.
"""
import os
import numpy as np
import ml_dtypes

import concourse.bass as bass
import concourse.bass_isa as bass_isa
import concourse.bacc as bacc
import concourse.tile as tile
import concourse.mybir as mybir
from concourse.bass import MemorySpace
from concourse.bass_utils import run_bass_kernel_spmd

BF16 = mybir.dt.bfloat16
F32 = mybir.dt.float32
AX = mybir.AxisListType.X
ALU = mybir.AluOpType
ACT = mybir.ActivationFunctionType

B, C, H, W = 8, 32, 128, 128
OUT = 32
HW = H * W          # 16384
HP, WP = 132, 132   # edge pad 2
N_CORES = 8
INV_N = 1.0 / (B * H * W)
EPS = 1e-5

NCHUNK_SEL = 8          # select-stage chunks (16 image rows each)
SEL_ROWS = H // NCHUNK_SEL


def _bf(x):
    return np.asarray(x, dtype=ml_dtypes.bfloat16)


def host_prep(inputs):
    """Pack weights for the device. Returns dict of np arrays (per-core
    replicated) keyed by dram tensor name."""
    wpk = np.asarray(inputs['w_pk'], np.float32)
    Wm = wpk.reshape(OUT, C, 16)

    # H-group lhsT [128=(g,c), 64]: cols 0:32 top (n=g), 32:64 bot (n=8+g)
    W_H = np.zeros((128, 64), np.float32)
    # V-group: right (n=4+g), left (n=12+g)
    W_V = np.zeros((128, 64), np.float32)
    for g in range(4):
        for c in range(C):
            W_H[g * 32 + c, 0:32] = Wm[:, c, g]
            W_H[g * 32 + c, 32:64] = Wm[:, c, 8 + g]
            W_V[g * 32 + c, 0:32] = Wm[:, c, 12 + g]   # left
            W_V[g * 32 + c, 32:64] = Wm[:, c, 4 + g]   # right

    # sum matmul: -identity blocks [128, 32]
    W_S = np.zeros((128, 32), np.float32)
    for blk in range(4):
        for o in range(32):
            W_S[blk * 32 + o, o] = -1.0

    # BandEst [96, 28]; col order ch*7+j, ch = (v0, hz0, v1, hz1)
    wv = np.asarray(inputs['w_vrt'], np.float32)    # (2,C,7,1)
    wh = np.asarray(inputs['w_hrz'], np.float32)    # (2,C,1,7)
    WB_full = np.zeros((C, 28), np.float32)
    for j in range(7):
        WB_full[:, 0 * 7 + j] = wv[0, :, j, 0]
        WB_full[:, 1 * 7 + j] = wh[0, :, 0, j]
        WB_full[:, 2 * 7 + j] = wv[1, :, j, 0]
        WB_full[:, 3 * 7 + j] = wh[1, :, 0, j]
    WB_hi = _bf(WB_full).astype(np.float32)
    WB_lo = WB_full - WB_hi
    # stack blocks pair with bs = [x_hi; x_hi; x_lo]:
    #   Whi@x_hi + Wlo@x_hi + Whi@x_lo  ~=  W@x to ~2^-17
    W_B = np.concatenate([WB_hi, WB_lo, WB_hi], axis=0)   # [96, 28]
    # center 1x1 conv folded into the same matmul: cols 28:60 = W_C acting
    # on x_hi (rows 0:32) + x_lo (rows 64:96)  ->  exact-x center plane
    Wc = Wm.sum(-1)                              # [o, c]
    W_B2 = np.zeros((96, 60), np.float32)
    W_B2[:, 0:28] = W_B
    for c in range(C):
        W_B2[c, 28:60] = Wc[:, c]
        W_B2[64 + c, 28:60] = Wc[:, c]
    bias60 = np.zeros((60, 1), np.float32)
    bias60[28:60, 0] = np.asarray(inputs['b_pk'], np.float32)

    W_G11 = Wm[:, :, 11].T.copy()               # [c, o]
    W_G15 = Wm[:, :, 15].T.copy()

    # broadcast lhsT: B4[d, 32d:32d+32] = 1 -> psum[128] = 4 dir blocks
    B4 = np.zeros((4, 128), np.float32)
    for d in range(4):
        B4[d, 32 * d:32 * d + 32] = 1.0

    gv = np.asarray(inputs['g_vrt'], np.float32)
    bev = np.asarray(inputs['be_vrt'], np.float32)
    gh = np.asarray(inputs['g_hrz'], np.float32)
    beh = np.asarray(inputs['be_hrz'], np.float32)
    # ch order (v0, hz0, v1, hz1)
    gamma = np.array([gv[0], gh[0], gv[1], gh[1]], np.float32)
    beta = np.array([bev[0], beh[0], bev[1], beh[1]], np.float32)
    bnp = np.concatenate([gamma, beta]).reshape(1, 8)

    bpk = np.asarray(inputs['b_pk'], np.float32).reshape(32, 1)

    return {
        'W_H': _bf(W_H), 'W_V': _bf(W_V), 'W_S': _bf(W_S),
        'W_B': _bf(W_B2), 'W_G11': _bf(W_G11), 'W_G15': _bf(W_G15),
        'bnp': bnp, 'bpk': bpk, 'B4': _bf(B4), 'bias60': bias60,
    }


def build(nc):
    x_d = nc.dram_tensor("x", [C, H, W], F32, kind="ExternalInput")
    wh_d = nc.dram_tensor("W_H", [128, 64], BF16, kind="ExternalInput")
    wv_d = nc.dram_tensor("W_V", [128, 64], BF16, kind="ExternalInput")
    ws_d = nc.dram_tensor("W_S", [128, 32], BF16, kind="ExternalInput")
    wb_d = nc.dram_tensor("W_B", [96, 60], BF16, kind="ExternalInput")
    wg11_d = nc.dram_tensor("W_G11", [32, 32], BF16, kind="ExternalInput")
    wg15_d = nc.dram_tensor("W_G15", [32, 32], BF16, kind="ExternalInput")
    bnp_d = nc.dram_tensor("bnp", [1, 8], F32, kind="ExternalInput")
    bpk_d = nc.dram_tensor("bpk", [32, 1], F32, kind="ExternalInput")
    b4_d = nc.dram_tensor("B4", [4, 128], BF16, kind="ExternalInput")
    bias60_d = nc.dram_tensor("bias60", [60, 1], F32, kind="ExternalInput")
    out_d = nc.dram_tensor("out", [OUT, H, W], F32, kind="ExternalOutput")

    with tile.TileContext(nc) as tc:
        _graph(nc, tc, x_d, wh_d, wv_d, ws_d, wb_d, wg11_d, wg15_d,
               bnp_d, bpk_d, b4_d, bias60_d, out_d)
    return nc


def _graph(nc, tc, x_d, wh_d, wv_d, ws_d, wb_d, wg11_d, wg15_d,
           bnp_d, bpk_d, b4_d, bias60_d, out_d):
    main = tc.alloc_tile_pool(name="main", bufs=1)
    dram = tc.alloc_tile_pool(name="dram", bufs=1, space="DRAM")
    late = tc.alloc_tile_pool(name="late", bufs=1)
    xp4hp = tc.alloc_tile_pool(name="xp4hp", bufs=1)
    xp4vp = tc.alloc_tile_pool(name="xp4vp", bufs=1)
    psum_acc = tc.alloc_tile_pool(name="psum_acc", bufs=2, space="PSUM")
    early = tc.alloc_tile_pool(name="early", bufs=1)
    psum_be = tc.alloc_tile_pool(name="psum_be", bufs=2, space="PSUM")
    chunkE = tc.alloc_tile_pool(name="chunkE", bufs=2)

    x_flat = x_d[:].rearrange("c h w -> c (h w)")

    # ---------------- weights ----------------
    w_h = main.tile([128, 64], BF16); nc.sync.dma_start(w_h[:], wh_d[:])
    w_v = main.tile([128, 64], BF16); nc.sync.dma_start(w_v[:], wv_d[:])
    w_s = main.tile([128, 32], BF16); nc.sync.dma_start(w_s[:], ws_d[:])
    w_b = main.tile([96, 60], BF16); nc.sync.dma_start(w_b[:], wb_d[:])
    w_g11 = main.tile([32, 32], BF16); nc.sync.dma_start(w_g11[:], wg11_d[:])
    w_g15 = main.tile([32, 32], BF16); nc.sync.dma_start(w_g15[:], wg15_d[:])
    bnp = main.tile([1, 8], F32); nc.sync.dma_start(bnp[:], bnp_d[:])
    bpk = main.tile([32, 1], F32); nc.sync.dma_start(bpk[:], bpk_d[:])
    b4 = main.tile([4, 128], BF16); nc.sync.dma_start(b4[:], b4_d[:])
    bias60 = main.tile([60, 1], F32); nc.sync.dma_start(bias60[:], bias60_d[:])

    # -------- BandEst stack + matmul + xpad interior, chunked --------
    xpad = early.tile([C, HP, WP], BF16)
    y_dram = dram.tile([60, HW], F32)
    NB = 8
    bchunk = HW // NB              # 2048 = 16 image rows
    for k in range(NB):
        sl = slice(k * bchunk, (k + 1) * bchunk)
        xs_c = chunkE.tile([C, bchunk], F32, tag="xs_c")
        nc.sync.dma_start(xs_c[:], x_flat[:, sl])
        bs_c = chunkE.tile([96, bchunk], BF16, tag="bs_c")
        nc.gpsimd.dma_start(bs_c[0:32, :], x_flat[:, sl])    # cast f32->bf16
        nc.gpsimd.dma_start(bs_c[32:64, :], x_flat[:, sl])
        subq = nc.vector if k % 2 == 0 else nc.gpsimd
        subq.tensor_tensor(bs_c[64:96, :], xs_c[:], bs_c[0:32, :],
                           ALU.subtract)
        # xpad interior rows for this chunk (sbuf->sbuf DMA, strided out)
        nc.sync.dma_start(
            xpad[:, 16 * k + 2:16 * k + 18, 2:130],
            bs_c[0:32, :].rearrange("c (r w) -> c r w", w=W))
        for i in range(bchunk // 1024):
            pb = psum_be.tile([60, 2, 512], F32, tag="pbe")
            for u in range(2):
                o0 = i * 1024 + u * 512
                nc.tensor.matmul(pb[:, u, :], w_b[:], bs_c[:, o0:o0 + 512],
                                 start=True, stop=True)
            yc = chunkE.tile([60, 1024], F32, tag="yc")
            nc.scalar.activation(yc[:], pb[:].rearrange("p a b -> p (a b)"),
                                 ACT.Identity, bias=bias60[:, 0:1])
            yq = nc.gpsimd if i == 0 else nc.sync
            o0 = k * bchunk + i * 1024
            yq.dma_start(y_dram[0:60, o0:o0 + 1024], yc[:])
    psum_be.release()
    chunkE.release()

    # plane transpose: y_dram rows -> [h=partition, m, w] planes; v-channel
    # row shifts baked into the DMAs, edges zeroed by one memset.
    ytt = early.tile([128, 28, W], F32)
    nc.gpsimd.memset(ytt[:], 0.0)
    _yq = [nc.scalar, nc.gpsimd]
    for ch in (0, 2):
        for j in range(7):
            m = ch * 7 + j
            s = j - 3
            lo, hi = max(0, -s), min(128, 128 - s)
            _yq[(ch + j) % 2].dma_start(
                ytt[lo:hi, m, :],
                y_dram[m:m + 1, (lo + s) * W:(hi + s) * W])
    nc.scalar.dma_start(ytt[:, 7:14, :],
                        y_dram[7:14, :].rearrange("m (h w) -> h m w", w=W))
    nc.gpsimd.dma_start(ytt[:, 21:28, :],
                        y_dram[21:28, :].rearrange("m (h w) -> h m w", w=W))

    # preact planes stacked [128, 4, 128]; ch order (v0=top, hz0=left,
    # v1=bot, hz1=right) -> dir stack (top, left, bot, right)
    preS = main.tile([128, 4, W], F32)
    for ch in range(4):
        nc.vector.tensor_copy(preS[:, ch, :], ytt[:, ch * 7 + 3, :])
        for j in [0, 1, 2, 4, 5, 6]:
            s = j - 3
            if ch in (0, 2):   # rows pre-shifted in the DMA: full add
                nc.vector.tensor_tensor(preS[:, ch, :], preS[:, ch, :],
                                        ytt[:, ch * 7 + j, :], ALU.add)
            else:              # horizontal: col (free) shifts
                lo, hi = max(0, -s), min(128, 128 - s)
                nc.vector.tensor_tensor(preS[:, ch, lo:hi], preS[:, ch, lo:hi],
                                        ytt[:, ch * 7 + j, lo + s:hi + s],
                                        ALU.add)

    # ---- BN stats early so the AllReduce overlaps the conv phase ----
    colsum = main.tile([128, 8], F32)
    sqt = main.tile([H, W], F32)
    for ch in range(4):
        nc.vector.tensor_reduce(colsum[:, ch:ch + 1], preS[:, ch, :], AX, ALU.add)
        nc.vector.tensor_tensor(sqt[:], preS[:, ch, :], preS[:, ch, :], ALU.mult)
        nc.vector.tensor_reduce(colsum[:, 4 + ch:5 + ch], sqt[:], AX, ALU.add)
    sums = main.tile([128, 8], F32)
    nc.gpsimd.partition_all_reduce(sums[:], colsum[:], 128,
                                   bass_isa.ReduceOp.add)

    # ---------------- padded image edges + XP4 (via DRAM) ----------------
    for dst, src_ in [(0, 2), (1, 2), (130, 129), (131, 129)]:
        nc.vector.tensor_copy(xpad[:, 2:130, dst:dst + 1],
                              xpad[:, 2:130, src_:src_ + 1])
    for dst, src_ in [(0, 2), (1, 2), (130, 129), (131, 129)]:
        nc.vector.tensor_copy(xpad[:, dst, :], xpad[:, src_, :])

    xpad_dram = dram.tile([C, HP * WP], BF16)
    nc.sync.dma_start(xpad_dram[:], xpad[:].rearrange("c a b -> c (a b)"))
    xp4h = xp4hp.tile([128, HP * WP], BF16)
    xp4v = xp4vp.tile([128, HP * WP], BF16)
    for g in range(4):
        nc.sync.dma_start(xp4h[32 * g:32 * g + 32, 0:HP * WP - g],
                          xpad_dram[:, g:HP * WP])
        nc.sync.dma_start(xp4v[32 * g:32 * g + 32, 0:HP * WP - g * WP],
                          xpad_dram[:, g * WP:HP * WP])
    vh = xp4h[:].rearrange("p (r c) -> p r c", c=WP)   # [128, 132, 132]
    vv = xp4v[:].rearrange("p (r c) -> p r c", c=WP)

    ar_in = dram.tile([1, 8], F32)
    ar_out = dram.tile([1, 8], F32)
    nc.gpsimd.dma_start(ar_in[:], sums[0:1, :])
    nc.gpsimd.collective_compute(
        "AllReduce", ALU.add,
        replica_groups=[list(range(N_CORES))],
        ins=[ar_in.opt()], outs=[ar_out.opt()],
    )
    gs = main.tile([1, 8], F32)
    nc.gpsimd.dma_start(gs[:], ar_out[:])

    early.release()
    psum_hv = tc.alloc_tile_pool(name="psum_hv", bufs=1, space="PSUM")

    # ---------------- directional convs -> F planes (bf16) ----------------
    # fhv 0:64 = F_H [64, 129, 128]: top(0:32) rr=F_top[max(rr-2,0)];
    #   bot(32:64) rr=F_bot[min(rr+4,131)], psum col w+1.
    #   select reads: P = [:, h+1, w], Q = [:, h, w]
    # fhv 64:128 = F_V [64, 128, 129]: left(0:32) cc=F_left[r+1,max(cc-2,0)];
    #   right(32:64) cc=F_right[r, min(cc+4,131)]
    #   select reads: P = [:, h, w+1], Q = [:, h, w]
    fhv = late.tile([128, 129 * 128], BF16)
    f_h = fhv[0:64].rearrange("p (r c) -> p r c", r=129)
    f_v = fhv[64:128].rearrange("p (r c) -> p r c", r=128)

    _ev_state = [0]
    def _evict(dst, srcv):
        e = _ev_state[0] % 2
        _ev_state[0] += 1
        if e == 0:
            nc.scalar.activation(dst, srcv, ACT.Identity)
        else:
            nc.vector.tensor_copy(dst, srcv)

    # interleave H and V 6-row chunks; psum [64, 2, 512] (bank-aligned halves)
    for k in range(22):
        r0 = 6 * k
        ph = psum_hv.tile([64, 2, 512], F32, tag="ph")
        for u in range(2):
            nc.tensor.matmul(ph[:, u, 0:387], w_h[:],
                             vh[:, r0 + 3 * u:r0 + 3 * u + 3, 0:129],
                             start=True, stop=True)
        phv = ph[:, :, 0:387].rearrange("p a (r c) -> p a r c", c=129)
        # top: rr = r+2, r in [0..126]
        lo, hi = r0, min(r0 + 6, 127)
        if lo < hi:
            for hf, a, b, oo in _psum_pieces(lo - r0, hi - r0):
                _evict(f_h[0:32, lo + oo + 2:lo + oo + 2 + (b - a), :],
                       phv[0:32, hf, a:b, 0:128])
        # bot: rr = r-4, r in [4..131]
        lo, hi = max(r0, 4), min(r0 + 6, 132)
        if lo < hi:
            for hf, a, b, oo in _psum_pieces(lo - r0, hi - r0):
                _evict(f_h[32:64, lo + oo - 4:lo + oo - 4 + (b - a), :],
                       phv[32:64, hf, a:b, 1:129])

        pvt = psum_hv.tile([64, 2, 512], F32, tag="pv")
        hp0 = 6 * k
        nrows = min(6, 129 - hp0)
        if nrows <= 0:
            continue
        for u in range(2):
            nr = min(3, 129 - hp0 - 3 * u)
            if nr > 0:
                nc.tensor.matmul(pvt[:, u, 0:nr * 132], w_v[:],
                                 vv[:, hp0 + 3 * u:hp0 + 3 * u + nr, 0:132],
                                 start=True, stop=True)
        pvv = pvt[:, :, 0:396].rearrange("p a (r c) -> p a r c", c=132)
        # left (0:32): r = hp-1, hp in [1..128]; cc = col+2 for cols 0..126
        lo, hi = max(hp0, 1), min(hp0 + nrows, 129)
        if lo < hi:
            for hf, a, b, oo in _psum_pieces(lo - hp0, hi - hp0):
                _evict(f_v[0:32, lo + oo - 1:lo + oo - 1 + (b - a), 2:129],
                       pvv[0:32, hf, a:b, 0:127])
        # right (32:64): r = hp in [0..127]; cc <- psum col cc+4
        lo, hi = hp0, min(hp0 + nrows, 128)
        if lo < hi:
            for hf, a, b, oo in _psum_pieces(lo - hp0, hi - hp0):
                _evict(f_v[32:64, lo + oo:lo + oo + (b - a), 0:128],
                       pvv[32:64, hf, a:b, 4:132])

    # one-time dup rows/cols (self copies after main fills)
    nc.vector.tensor_copy(f_h[0:32, 0, :], f_h[0:32, 2, :])
    nc.vector.tensor_copy(f_h[0:32, 1, :], f_h[0:32, 2, :])
    nc.vector.tensor_copy(f_h[32:64, 128, :], f_h[32:64, 127, :])
    nc.vector.tensor_copy(f_v[0:32, :, 0:1], f_v[0:32, :, 2:3])
    nc.vector.tensor_copy(f_v[0:32, :, 1:2], f_v[0:32, :, 2:3])
    nc.vector.tensor_copy(f_v[32:64, :, 128:129], f_v[32:64, :, 127:128])

    psum_hv.release()
    psum_bc = tc.alloc_tile_pool(name="psum_bc", bufs=2, space="PSUM")
    xp4vp.release()

    # ---------------- rb corner strip matmuls ----------------
    g11rhs = main.tile([32, 132], BF16)
    nc.sync.dma_start(g11rhs[:], vh[0:32, :, 131])
    pg = psum_acc.tile([32, 512], F32, tag="acc")
    nc.tensor.matmul(pg[0:32, 0:132], w_g11[:], g11rhs[:],
                     start=True, stop=True)
    g11e = main.tile([32, 134], F32)
    nc.scalar.activation(g11e[:, 0:132], pg[0:32, 0:132], ACT.Identity)
    nc.scalar.activation(g11e[:, 132:133], pg[0:32, 131:132], ACT.Identity)
    nc.scalar.activation(g11e[:, 133:134], pg[0:32, 131:132], ACT.Identity)
    pg2 = psum_acc.tile([32, 512], F32, tag="acc")
    nc.tensor.matmul(pg2[0:32, 0:132], w_g15[:], vh[0:32, 131, 0:132],
                     start=True, stop=True)
    g15e = main.tile([32, 133], F32)
    nc.scalar.activation(g15e[:, 1:133], pg2[0:32, 0:132], ACT.Identity)
    nc.scalar.activation(g15e[:, 0:1], pg2[0:32, 0:1], ACT.Identity)
    xp4hp.release()

    # ---------------- BN consts from the early AllReduce ----------------
    # tile_wait_until: the AllReduce lands late — keep these ops from being
    # scheduled ahead of the conv-phase evictions in the engine queues
    with tc.tile_wait_until(0.21):
        mu = main.tile([1, 4], F32)
        nc.vector.tensor_scalar(mu[:], gs[:, 0:4], INV_N, None, ALU.mult)
        ex2 = main.tile([1, 4], F32)
        nc.vector.tensor_scalar(ex2[:], gs[:, 4:8], INV_N, None, ALU.mult)
        var = main.tile([1, 4], F32)
        nc.vector.tensor_tensor(var[:], mu[:], mu[:], ALU.mult)
        nc.vector.tensor_tensor(var[:], ex2[:], var[:], ALU.subtract)
        nc.vector.tensor_scalar(var[:], var[:], EPS, None, ALU.add)
        sd = main.tile([1, 4], F32)
        nc.scalar.sqrt(sd[:], var[:])
        rsq = main.tile([1, 4], F32)
        nc.vector.reciprocal(rsq[:], sd[:])
        zscale = main.tile([1, 4], F32)
        nc.vector.tensor_tensor(zscale[:], rsq[:], bnp[:, 0:4], ALU.mult)
        zbias = main.tile([1, 4], F32)
        nc.vector.tensor_tensor(zbias[:], mu[:], zscale[:], ALU.mult)
        nc.vector.tensor_tensor(zbias[:], bnp[:, 4:8], zbias[:], ALU.subtract)
        cons = main.tile([128, 8], F32)   # bcast: zscale 0:4, zbias 4:8
        nc.gpsimd.partition_broadcast(cons[:, 0:4], zscale[:])
        nc.gpsimd.partition_broadcast(cons[:, 4:8], zbias[:])

    # ------------- coefficient planes, dir-stacked [128, 4, 128] ----------
    # dir stack order: (top, left, bot, right); minus dirs = 0:2, plus = 2:4
    coeffp = tc.alloc_tile_pool(name="coeffp", bufs=1)
    ih = main.tile([H, W], mybir.dt.int32)
    nc.gpsimd.iota(ih[:], pattern=[[0, W]], base=0, channel_multiplier=1)
    iw = main.tile([H, W], mybir.dt.int32)
    nc.gpsimd.iota(iw[:], pattern=[[1, W]], base=0, channel_multiplier=0)
    idxb1 = coeffp.tile([128, 4, W], F32)   # IDX - 1 (minus) / IDX + 4 (plus)
    idxb2 = coeffp.tile([128, 4, W], F32)   # IDX (minus) / IDX + 4 (plus)
    for col, (srci, o1, o2) in enumerate([(0, -1.0, 0.0), (1, -1.0, 0.0),
                                          (0, 4.0, 4.0), (1, 4.0, 4.0)]):
        srct = ih if srci == 0 else iw
        nc.vector.tensor_scalar(idxb1[:, col, :], srct[:], o1, None, ALU.add)
        nc.vector.tensor_scalar(idxb2[:, col, :], srct[:], o2, None, ALU.add)

    z = coeffp.tile([128, 4, W], F32)
    for ch in range(4):
        nc.vector.tensor_scalar(z[:, ch, :], preS[:, ch, :],
                                cons[:, ch:ch + 1], cons[:, 4 + ch:5 + ch],
                                ALU.mult, ALU.add)
    gb = coeffp.tile([128, 4, W], F32)
    nc.scalar.activation(gb[:].rearrange("p a b -> p (a b)"),
                         z[:].rearrange("p a b -> p (a b)"), ACT.Sigmoid)
    nc.vector.tensor_scalar(gb[:], gb[:], 2.0, None, ALU.mult)
    m2 = coeffp.tile([128, 4, W], F32)
    nc.vector.tensor_scalar(m2[:], z[:], 0.0, None, ALU.is_gt)

    q = coeffp.tile([128, 4, W], F32)
    nc.vector.tensor_tensor(q[:, 0:2, :], idxb1[:, 0:2, :], m2[:, 0:2, :],
                            ALU.subtract)
    nc.vector.tensor_tensor(q[:, 2:4, :], idxb1[:, 2:4, :], m2[:, 2:4, :],
                            ALU.add)
    nc.vector.tensor_scalar(q[:, 0:2, :], q[:, 0:2, :], 0.0, None, ALU.max)
    nc.vector.tensor_scalar(q[:, 2:4, :], q[:, 2:4, :], 131.0, None, ALU.min)
    pcl = coeffp.tile([128, 4, W], F32)
    nc.vector.tensor_tensor(pcl[:, 0:2, :], idxb2[:, 0:2, :], gb[:, 0:2, :],
                            ALU.subtract)
    nc.vector.tensor_tensor(pcl[:, 2:4, :], idxb2[:, 2:4, :], gb[:, 2:4, :],
                            ALU.add)
    nc.vector.tensor_scalar(pcl[:, 0:2, :], pcl[:, 0:2, :], 0.0, None, ALU.max)
    nc.vector.tensor_scalar(pcl[:, 2:4, :], pcl[:, 2:4, :], 131.0, None,
                            ALU.min)
    wlt = coeffp.tile([128, 4, W], F32)
    nc.vector.tensor_tensor(wlt[:], q[:], pcl[:], ALU.subtract)
    nc.vector.tensor_scalar(wlt[:], wlt[:], 1.0, None, ALU.add)
    # cq = wlt * [m2, m2, 1-m2, 1-m2]; cp = wlt - cq
    bmul = coeffp.tile([128, 4, W], F32)
    nc.vector.tensor_tensor(bmul[:], wlt[:], m2[:], ALU.mult)
    cqS = coeffp.tile([128, 4, W], BF16)
    nc.vector.tensor_copy(cqS[:, 0:2, :], bmul[:, 0:2, :])
    nc.vector.tensor_tensor(cqS[:, 2:4, :], wlt[:, 2:4, :], bmul[:, 2:4, :],
                            ALU.subtract)
    cpS = coeffp.tile([128, 4, W], BF16)
    nc.vector.tensor_tensor(cpS[:], wlt[:], cqS[:], ALU.subtract)

    # ab_dram rows (order = S partition groups): cp: [top, bot, left, right]
    # cq rows 4:8 same order. Stack cols (t,l,b,r) -> rows via 2 DMAs each.
    ab_dram = dram.tile([8, HW], BF16)
    for base, tsrc in ((0, cpS), (4, cqS)):
        nc.sync.dma_start(
            ab_dram[base:base + 2, :].rearrange("d (h w) -> h d w", w=W),
            tsrc[:, 0:4:2, :])          # cols (top, bot) -> rows +0,+1
        nc.gpsimd.dma_start(
            ab_dram[base + 2:base + 4, :].rearrange("d (h w) -> h d w", w=W),
            tsrc[:, 1:4:2, :])          # cols (left, right) -> rows +2,+3


    # ---------------- rb corner strip coefficients ----------------
    # bottom strip at (h, 127): dir col 2; [128,1] partition-major
    ihf1 = main.tile([128, 1], F32)
    nc.vector.tensor_copy(ihf1[:], ih[:, 0:1])
    m2c = main.tile([128, 1], F32)
    nc.vector.tensor_copy(m2c[:], m2[:, 2, 127:128])
    gbc = main.tile([128, 1], F32)
    nc.vector.tensor_copy(gbc[:], gb[:, 2, 127:128])
    qs = main.tile([128, 1], F32)
    nc.vector.tensor_tensor(qs[:], ihf1[:], m2c[:], ALU.add)
    nc.vector.tensor_scalar(qs[:], qs[:], 5.0, 131.0, ALU.add, ALU.min)
    ps_ = main.tile([128, 1], F32)
    nc.vector.tensor_tensor(ps_[:], ihf1[:], gbc[:], ALU.add)
    nc.vector.tensor_scalar(ps_[:], ps_[:], 4.0, 131.0, ALU.add, ALU.min)
    wrb = main.tile([128, 1], F32)
    nc.vector.tensor_tensor(wrb[:], qs[:], ps_[:], ALU.subtract)
    nc.vector.tensor_scalar(wrb[:], wrb[:], 1.0, None, ALU.subtract)
    cbs = main.tile([128, 1], F32)
    nc.vector.tensor_tensor(cbs[:], wrb[:], m2c[:], ALU.mult)
    cas = main.tile([128, 1], F32)
    nc.vector.tensor_tensor(cas[:], wrb[:], cbs[:], ALU.subtract)
    strip_dram = dram.tile([4, 128], F32)
    nc.sync.dma_start(strip_dram[0:1, :], cas[:])
    nc.sync.dma_start(strip_dram[1:2, :], cbs[:])

    # left strip at (127, w): dir col 1; [1, 128] via DMA off partition 127
    m2r = main.tile([1, 128], F32)
    nc.sync.dma_start(m2r[:], m2[127:128, 1, :])
    gbr = main.tile([1, 128], F32)
    nc.sync.dma_start(gbr[:], gb[127:128, 1, :])
    iwf1 = main.tile([1, 128], F32)
    nc.vector.tensor_copy(iwf1[:], iw[0:1, :])
    qs2 = main.tile([1, 128], F32)
    nc.vector.tensor_tensor(qs2[:], iwf1[:], m2r[:], ALU.subtract)
    nc.vector.tensor_scalar(qs2[:], qs2[:], 0.0, None, ALU.max)
    ps2 = main.tile([1, 128], F32)
    nc.vector.tensor_tensor(ps2[:], iwf1[:], gbr[:], ALU.subtract)
    nc.vector.tensor_scalar(ps2[:], ps2[:], 0.0, None, ALU.max)
    wrb2 = main.tile([1, 128], F32)
    nc.vector.tensor_tensor(wrb2[:], qs2[:], ps2[:], ALU.subtract)
    nc.vector.tensor_scalar(wrb2[:], wrb2[:], 1.0, None, ALU.subtract)
    cb2 = main.tile([1, 128], F32)
    nc.vector.tensor_tensor(cb2[:], wrb2[:], m2r[:], ALU.mult)
    ca2 = main.tile([1, 128], F32)
    nc.vector.tensor_tensor(ca2[:], wrb2[:], cb2[:], ALU.subtract)
    nc.sync.dma_start(strip_dram[2:3, :], ca2[:])
    nc.sync.dma_start(strip_dram[3:4, :], cb2[:])

    strips = main.tile([32, 4, 128], F32)
    nc.sync.dma_start(strips[:],
                      strip_dram[:].unsqueeze(0).broadcast_to([32, 4, 128]))
    t1 = main.tile([32, 128], F32)
    nc.vector.tensor_tensor(t1[:], strips[:, 0, :], g11e[:, 5:133], ALU.mult)
    t2 = main.tile([32, 128], F32)
    nc.vector.tensor_tensor(t2[:], strips[:, 1, :], g11e[:, 6:134], ALU.mult)
    nc.vector.tensor_tensor(t1[:], t1[:], t2[:], ALU.add)
    t3 = main.tile([32, 128], F32)
    nc.vector.tensor_tensor(t3[:], strips[:, 2, :], g15e[:, 1:129], ALU.mult)
    t4 = main.tile([32, 128], F32)
    nc.vector.tensor_tensor(t4[:], strips[:, 3, :], g15e[:, 0:128], ALU.mult)
    nc.vector.tensor_tensor(t3[:], t3[:], t4[:], ALU.add)

    # ---------------- select stage + final matmuls ----------------
    # Software-pipelined: stage A(k) (bc matmuls -> bf16 evict -> 2x DVE
    # multiplies) is emitted before stage B(k-1) (acc matmuls + out) so the
    # Tensor stream always has ready work.
    coeffp.release()
    chunkL2 = tc.alloc_tile_pool(name="chunkL2", bufs=3)
    out_flat = out_d[:].rearrange("o h w -> o (h w)")

    def sel_stageA(kc):
        r0 = kc * SEL_ROWS
        pix0 = r0 * W
        npix = SEL_ROWS * W                      # 2048
        s1 = chunkL2.tile([128, SEL_ROWS, W], BF16, tag="s1")   # P-stack
        s2 = chunkL2.tile([128, SEL_ROWS, W], BF16, tag="s2")   # Q-stack
        ab_cp = chunkL2.tile([4, npix], BF16, tag="ab_cp")
        nc.gpsimd.dma_start(ab_cp[:], ab_dram[0:4, pix0:pix0 + npix])
        ab_cq = chunkL2.tile([4, npix], BF16, tag="ab_cq")
        nc.gpsimd.dma_start(ab_cq[:], ab_dram[4:8, pix0:pix0 + npix])
        cen = chunkL2.tile([OUT, npix], F32, tag="cen")
        nc.gpsimd.dma_start(cen[:], y_dram[28:60, pix0:pix0 + npix])
        for hf in range(2):
            rr = r0 + 8 * hf            # image rows rr..rr+8 in this half
            rel = 8 * hf
            cpb = chunkL2.tile([128, 8, W], BF16, tag="cpb")
            cqb = chunkL2.tile([128, 8, W], BF16, tag="cqb")
            for u in range(2):
                o_l = 1024 * hf + 512 * u
                pbc = psum_bc.tile([128, 2, 512], F32, tag="pbc")
                nc.tensor.matmul(pbc[:, 0, :], b4[:],
                                 ab_cp[:, o_l:o_l + 512],
                                 start=True, stop=True)
                nc.tensor.matmul(pbc[:, 1, :], b4[:],
                                 ab_cq[:, o_l:o_l + 512],
                                 start=True, stop=True)
                # evict to bf16 SBUF so the multiplies hit DVE 2x mode
                nc.scalar.activation(
                    cpb[:, 4 * u:4 * u + 4, :].rearrange("p r w -> p (r w)"),
                    pbc[:, 0, :], ACT.Identity)
                nc.scalar.activation(
                    cqb[:, 4 * u:4 * u + 4, :].rearrange("p r w -> p (r w)"),
                    pbc[:, 1, :], ACT.Identity)
            nc.vector.tensor_tensor(s1[0:64, rel:rel + 8, :], cpb[0:64],
                                    f_h[:, rr + 1:rr + 9, :], ALU.mult)
            nc.vector.tensor_tensor(s2[0:64, rel:rel + 8, :], cqb[0:64],
                                    f_h[:, rr:rr + 8, :], ALU.mult)
            nc.vector.tensor_tensor(s1[64:128, rel:rel + 8, :], cpb[64:128],
                                    f_v[:, rr:rr + 8, 1:129], ALU.mult)
            nc.vector.tensor_tensor(s2[64:128, rel:rel + 8, :], cqb[64:128],
                                    f_v[:, rr:rr + 8, 0:128], ALU.mult)
        return s1, s2, cen

    def sel_stageB(kc, s1, s2, cen):
        r0 = kc * SEL_ROWS
        pix0 = r0 * W
        npix = SEL_ROWS * W
        out_c = chunkL2.tile([OUT, SEL_ROWS, W], F32, tag="out_c")
        s1v = s1[:].rearrange("p r w -> p (r w)")
        s2v = s2[:].rearrange("p r w -> p (r w)")
        for i in range(npix // 512):
            acc = psum_acc.tile([32, 512], F32, tag="acc")
            nc.tensor.matmul(acc[:], w_s[:], s1v[:, 512 * i:512 * (i + 1)],
                             start=True, stop=False)
            nc.tensor.matmul(acc[:], w_s[:], s2v[:, 512 * i:512 * (i + 1)],
                             start=False, stop=True)
            nc.vector.tensor_tensor(
                out_c[:, 4 * i:4 * i + 4, :].rearrange("p r w -> p (r w)"),
                acc[:], cen[:, 512 * i:512 * (i + 1)], ALU.add)
        nc.vector.tensor_tensor(out_c[:, :, 127], out_c[:, :, 127],
                                t1[:, r0:r0 + SEL_ROWS], ALU.add)
        if kc == NCHUNK_SEL - 1:
            nc.vector.tensor_tensor(out_c[:, SEL_ROWS - 1, :],
                                    out_c[:, SEL_ROWS - 1, :], t3[:], ALU.add)
        nc.sync.dma_start(out_flat[:, pix0:pix0 + npix],
                          out_c[:].rearrange("o r w -> o (r w)"))

    prev = sel_stageA(0)
    for kc in range(1, NCHUNK_SEL):
        cur = sel_stageA(kc)
        sel_stageB(kc - 1, *prev)
        prev = cur
    sel_stageB(NCHUNK_SEL - 1, *prev)

    chunkL2.release()
    late.release()
    psum_bc.release()
    psum_acc.release()
    dram.release()
    main.release()


def _psum_pieces(rlo, rhi):
    """Split psum row range [rlo, rhi) in [0,6) into per-half pieces.

    Returns list of (half, half_rlo, half_rhi, out_row_offset_from_rlo)."""
    assert 0 <= rlo < rhi <= 6
    pieces = []
    if rlo < 3:
        e = min(rhi, 3)
        pieces.append((0, rlo, e, 0))
    if rhi > 3:
        s = max(rlo, 3)
        pieces.append((1, s - 3, rhi - 3, s - rlo))
    return pieces


_CACHED = {}


def _get_nc():
    if 'nc' not in _CACHED:
        nc = bacc.Bacc(None, target_bir_lowering=False)
        build(nc)
        nc.compile()
        _CACHED['nc'] = nc
    return _CACHED['nc']


def kernel(**inputs):
    nc = _get_nc()
    wd = host_prep(inputs)
    x = np.asarray(inputs['x'], np.float32)
    in_maps = []
    for i in range(N_CORES):
        m = {'x': np.ascontiguousarray(x[i])}
        m.update(wd)
        in_maps.append(m)
    res = run_bass_kernel_spmd(nc, in_maps, core_ids=list(range(N_CORES)))
    outs = [res.results[i]['out'] for i in range(N_CORES)]
    return np.stack(outs, axis=0)


if __name__ == '__main__':
    nc = _get_nc()
    print("build+compile OK")



# revision 38
# speedup vs baseline: 1.1434x; 1.0472x over previous
"""AdaPeakConv2D Trainium2 kernel — 8-core data parallel over batch.

Self-contained: hardcodes shapes B=8, C=32, H=W=128, OUT=32, KH=KW=4.

Decomposition (validated against the reference in numpy):
  out = center(1x1 conv) + b_pk
        - sum_d [ cP_d * F_d(P-shift) + cQ_d * F_d(Q-shift) ]
        - rb corner strips (w=127 bottom / h=127 left)
  - BandEst preact via one K=96 matmul (bf16 hi/lo split for ~fp32
    accuracy) + zero-pad shifted adds; BN stats via a 16-float AllReduce.
  - Directional 4-tap convs as two K=128 tap-packed matmuls (M=64) over
    shifted bf16 copies of the edge-padded image (XP4_H / XP4_V).
  - 2-way integer-shift select folded into broadcast coefficient planes.
  - Final combine: center matmul + two K=128 "sum" matmuls in PSUM.
"""
import os
import numpy as np
import ml_dtypes

import concourse.bass as bass
import concourse.bass_isa as bass_isa
import concourse.bacc as bacc
import concourse.tile as tile
import concourse.mybir as mybir
from concourse.bass import MemorySpace
from concourse.bass_utils import run_bass_kernel_spmd

BF16 = mybir.dt.bfloat16
F32 = mybir.dt.float32
AX = mybir.AxisListType.X
ALU = mybir.AluOpType
ACT = mybir.ActivationFunctionType

B, C, H, W = 8, 32, 128, 128
OUT = 32
HW = H * W          # 16384
HP, WP = 132, 132   # edge pad 2
N_CORES = 8
INV_N = 1.0 / (B * H * W)
EPS = 1e-5

NCHUNK_SEL = 8          # select-stage chunks (16 image rows each)
SEL_ROWS = H // NCHUNK_SEL


def _bf(x):
    return np.asarray(x, dtype=ml_dtypes.bfloat16)


def host_prep(inputs):
    """Pack weights for the device. Returns dict of np arrays (per-core
    replicated) keyed by dram tensor name."""
    wpk = np.asarray(inputs['w_pk'], np.float32)
    Wm = wpk.reshape(OUT, C, 16)

    # H-group lhsT [128=(g,c), 64]: cols 0:32 top (n=g), 32:64 bot (n=8+g)
    W_H = np.zeros((128, 64), np.float32)
    # V-group: right (n=4+g), left (n=12+g)
    W_V = np.zeros((128, 64), np.float32)
    for g in range(4):
        for c in range(C):
            W_H[g * 32 + c, 0:32] = Wm[:, c, g]
            W_H[g * 32 + c, 32:64] = Wm[:, c, 8 + g]
            W_V[g * 32 + c, 0:32] = Wm[:, c, 12 + g]   # left
            W_V[g * 32 + c, 32:64] = Wm[:, c, 4 + g]   # right

    # sum matmul: -identity blocks [128, 32]
    W_S = np.zeros((128, 32), np.float32)
    for blk in range(4):
        for o in range(32):
            W_S[blk * 32 + o, o] = -1.0

    # BandEst [96, 28]; col order ch*7+j, ch = (v0, hz0, v1, hz1)
    wv = np.asarray(inputs['w_vrt'], np.float32)    # (2,C,7,1)
    wh = np.asarray(inputs['w_hrz'], np.float32)    # (2,C,1,7)
    WB_full = np.zeros((C, 28), np.float32)
    for j in range(7):
        WB_full[:, 0 * 7 + j] = wv[0, :, j, 0]
        WB_full[:, 1 * 7 + j] = wh[0, :, 0, j]
        WB_full[:, 2 * 7 + j] = wv[1, :, j, 0]
        WB_full[:, 3 * 7 + j] = wh[1, :, 0, j]
    WB_hi = _bf(WB_full).astype(np.float32)
    WB_lo = WB_full - WB_hi
    # stack blocks pair with bs = [x_hi; x_hi; x_lo]:
    #   Whi@x_hi + Wlo@x_hi + Whi@x_lo  ~=  W@x to ~2^-17
    W_B = np.concatenate([WB_hi, WB_lo, WB_hi], axis=0)   # [96, 28]
    # center 1x1 conv folded into the same matmul: cols 28:60 = W_C acting
    # on x_hi (rows 0:32) + x_lo (rows 64:96)  ->  exact-x center plane
    Wc = Wm.sum(-1)                              # [o, c]
    W_B2 = np.zeros((96, 60), np.float32)
    W_B2[:, 0:28] = W_B
    for c in range(C):
        W_B2[c, 28:60] = Wc[:, c]
        W_B2[64 + c, 28:60] = Wc[:, c]
    bias60 = np.zeros((60, 1), np.float32)
    bias60[28:60, 0] = np.asarray(inputs['b_pk'], np.float32)

    W_G11 = Wm[:, :, 11].T.copy()               # [c, o]
    W_G15 = Wm[:, :, 15].T.copy()

    # broadcast lhsT: B4[d, 32d:32d+32] = 1 -> psum[128] = 4 dir blocks
    B4 = np.zeros((4, 128), np.float32)
    for d in range(4):
        B4[d, 32 * d:32 * d + 32] = 1.0

    gv = np.asarray(inputs['g_vrt'], np.float32)
    bev = np.asarray(inputs['be_vrt'], np.float32)
    gh = np.asarray(inputs['g_hrz'], np.float32)
    beh = np.asarray(inputs['be_hrz'], np.float32)
    # ch order (v0, hz0, v1, hz1)
    gamma = np.array([gv[0], gh[0], gv[1], gh[1]], np.float32)
    beta = np.array([bev[0], beh[0], bev[1], beh[1]], np.float32)
    bnp = np.concatenate([gamma, beta]).reshape(1, 8)

    bpk = np.asarray(inputs['b_pk'], np.float32).reshape(32, 1)

    return {
        'W_H': _bf(W_H), 'W_V': _bf(W_V), 'W_S': _bf(W_S),
        'W_B': _bf(W_B2), 'W_G11': _bf(W_G11), 'W_G15': _bf(W_G15),
        'bnp': bnp, 'bpk': bpk, 'B4': _bf(B4), 'bias60': bias60,
    }


def build(nc):
    x_d = nc.dram_tensor("x", [C, H, W], F32, kind="ExternalInput")
    wh_d = nc.dram_tensor("W_H", [128, 64], BF16, kind="ExternalInput")
    wv_d = nc.dram_tensor("W_V", [128, 64], BF16, kind="ExternalInput")
    ws_d = nc.dram_tensor("W_S", [128, 32], BF16, kind="ExternalInput")
    wb_d = nc.dram_tensor("W_B", [96, 60], BF16, kind="ExternalInput")
    wg11_d = nc.dram_tensor("W_G11", [32, 32], BF16, kind="ExternalInput")
    wg15_d = nc.dram_tensor("W_G15", [32, 32], BF16, kind="ExternalInput")
    bnp_d = nc.dram_tensor("bnp", [1, 8], F32, kind="ExternalInput")
    bpk_d = nc.dram_tensor("bpk", [32, 1], F32, kind="ExternalInput")
    b4_d = nc.dram_tensor("B4", [4, 128], BF16, kind="ExternalInput")
    bias60_d = nc.dram_tensor("bias60", [60, 1], F32, kind="ExternalInput")
    out_d = nc.dram_tensor("out", [OUT, H, W], F32, kind="ExternalOutput")

    with tile.TileContext(nc) as tc:
        _graph(nc, tc, x_d, wh_d, wv_d, ws_d, wb_d, wg11_d, wg15_d,
               bnp_d, bpk_d, b4_d, bias60_d, out_d)
    return nc


def _graph(nc, tc, x_d, wh_d, wv_d, ws_d, wb_d, wg11_d, wg15_d,
           bnp_d, bpk_d, b4_d, bias60_d, out_d):
    main = tc.alloc_tile_pool(name="main", bufs=1)
    dram = tc.alloc_tile_pool(name="dram", bufs=1, space="DRAM")
    late = tc.alloc_tile_pool(name="late", bufs=1)
    xp4hp = tc.alloc_tile_pool(name="xp4hp", bufs=1)
    xp4vp = tc.alloc_tile_pool(name="xp4vp", bufs=1)
    psum_acc = tc.alloc_tile_pool(name="psum_acc", bufs=2, space="PSUM")
    early = tc.alloc_tile_pool(name="early", bufs=1)
    psum_be = tc.alloc_tile_pool(name="psum_be", bufs=2, space="PSUM")
    chunkE = tc.alloc_tile_pool(name="chunkE", bufs=2)

    x_flat = x_d[:].rearrange("c h w -> c (h w)")

    # ---------------- weights ----------------
    w_h = main.tile([128, 64], BF16); nc.sync.dma_start(w_h[:], wh_d[:])
    w_v = main.tile([128, 64], BF16); nc.sync.dma_start(w_v[:], wv_d[:])
    w_s = main.tile([128, 32], BF16); nc.sync.dma_start(w_s[:], ws_d[:])
    w_b = main.tile([96, 60], BF16); nc.sync.dma_start(w_b[:], wb_d[:])
    w_g11 = main.tile([32, 32], BF16); nc.sync.dma_start(w_g11[:], wg11_d[:])
    w_g15 = main.tile([32, 32], BF16); nc.sync.dma_start(w_g15[:], wg15_d[:])
    bnp = main.tile([1, 8], F32); nc.sync.dma_start(bnp[:], bnp_d[:])
    bpk = main.tile([32, 1], F32); nc.sync.dma_start(bpk[:], bpk_d[:])
    b4 = main.tile([4, 128], BF16); nc.sync.dma_start(b4[:], b4_d[:])
    bias60 = main.tile([60, 1], F32); nc.sync.dma_start(bias60[:], bias60_d[:])

    # -------- BandEst stack + matmul + xpad interior, chunked --------
    xpad = early.tile([C, HP, WP], BF16)
    y_dram = dram.tile([60, HW], F32)
    NB = 8
    bchunk = HW // NB              # 2048 = 16 image rows
    for k in range(NB):
        sl = slice(k * bchunk, (k + 1) * bchunk)
        xs_c = chunkE.tile([C, bchunk], F32, tag="xs_c")
        nc.sync.dma_start(xs_c[:], x_flat[:, sl])
        bs_c = chunkE.tile([96, bchunk], BF16, tag="bs_c")
        nc.gpsimd.dma_start(bs_c[0:32, :], x_flat[:, sl])    # cast f32->bf16
        nc.gpsimd.dma_start(bs_c[32:64, :], x_flat[:, sl])
        nc.vector.tensor_tensor(bs_c[64:96, :], xs_c[:], bs_c[0:32, :],
                                ALU.subtract)
        # xpad interior rows for this chunk (sbuf->sbuf DMA, strided out)
        nc.sync.dma_start(
            xpad[:, 16 * k + 2:16 * k + 18, 2:130],
            bs_c[0:32, :].rearrange("c (r w) -> c r w", w=W))
        for i in range(bchunk // 1024):
            pb = psum_be.tile([60, 2, 512], F32, tag="pbe")
            for u in range(2):
                o0 = i * 1024 + u * 512
                nc.tensor.matmul(pb[:, u, :], w_b[:], bs_c[:, o0:o0 + 512],
                                 start=True, stop=True)
            yc = chunkE.tile([60, 1024], F32, tag="yc")
            nc.scalar.activation(yc[:], pb[:].rearrange("p a b -> p (a b)"),
                                 ACT.Identity, bias=bias60[:, 0:1])
            yq = nc.gpsimd if i == 0 else nc.sync
            o0 = k * bchunk + i * 1024
            yq.dma_start(y_dram[0:60, o0:o0 + 1024], yc[:])
    psum_be.release()
    chunkE.release()

    # plane transpose: y_dram rows -> [h=partition, m, w] planes; v-channel
    # row shifts baked into the DMAs, edges zeroed by one memset.
    ytt = early.tile([128, 28, W], F32)
    nc.gpsimd.memset(ytt[:], 0.0)
    for ch in (0, 2):
        for j in range(7):
            m = ch * 7 + j
            s = j - 3
            lo, hi = max(0, -s), min(128, 128 - s)
            nc.gpsimd.dma_start(
                ytt[lo:hi, m, :],
                y_dram[m:m + 1, (lo + s) * W:(hi + s) * W])
    nc.gpsimd.dma_start(ytt[:, 7:14, :],
                        y_dram[7:14, :].rearrange("m (h w) -> h m w", w=W))
    nc.gpsimd.dma_start(ytt[:, 21:28, :],
                        y_dram[21:28, :].rearrange("m (h w) -> h m w", w=W))

    # preact planes stacked [128, 4, 128]; ch order (v0=top, hz0=left,
    # v1=bot, hz1=right) -> dir stack (top, left, bot, right)
    preS = main.tile([128, 4, W], F32)
    for ch in range(4):
        nc.vector.tensor_copy(preS[:, ch, :], ytt[:, ch * 7 + 3, :])
        for j in [0, 1, 2, 4, 5, 6]:
            s = j - 3
            if ch in (0, 2):   # rows pre-shifted in the DMA: full add
                nc.vector.tensor_tensor(preS[:, ch, :], preS[:, ch, :],
                                        ytt[:, ch * 7 + j, :], ALU.add)
            else:              # horizontal: col (free) shifts
                lo, hi = max(0, -s), min(128, 128 - s)
                nc.vector.tensor_tensor(preS[:, ch, lo:hi], preS[:, ch, lo:hi],
                                        ytt[:, ch * 7 + j, lo + s:hi + s],
                                        ALU.add)

    # ---- BN stats early so the AllReduce overlaps the conv phase ----
    colsum = main.tile([128, 8], F32)
    sqt = main.tile([H, W], F32)
    for ch in range(4):
        nc.vector.tensor_reduce(colsum[:, ch:ch + 1], preS[:, ch, :], AX, ALU.add)
        nc.vector.tensor_tensor(sqt[:], preS[:, ch, :], preS[:, ch, :], ALU.mult)
        nc.vector.tensor_reduce(colsum[:, 4 + ch:5 + ch], sqt[:], AX, ALU.add)
    sums = main.tile([128, 8], F32)
    nc.gpsimd.partition_all_reduce(sums[:], colsum[:], 128,
                                   bass_isa.ReduceOp.add)

    # ---------------- padded image edges + XP4 (via DRAM) ----------------
    for dst, src_ in [(0, 2), (1, 2), (130, 129), (131, 129)]:
        nc.vector.tensor_copy(xpad[:, 2:130, dst:dst + 1],
                              xpad[:, 2:130, src_:src_ + 1])
    for dst, src_ in [(0, 2), (1, 2), (130, 129), (131, 129)]:
        nc.vector.tensor_copy(xpad[:, dst, :], xpad[:, src_, :])

    xp4h = xp4hp.tile([128, HP * WP], BF16)
    xp4v = xp4vp.tile([128, HP * WP], BF16)
    xpad_f = xpad[:].rearrange("c a b -> c (a b)")
    for g in range(4):
        nc.sync.dma_start(xp4h[32 * g:32 * g + 32, 0:HP * WP - g],
                          xpad_f[:, g:HP * WP])
        nc.sync.dma_start(xp4v[32 * g:32 * g + 32, 0:HP * WP - g * WP],
                          xpad_f[:, g * WP:HP * WP])
    vh = xp4h[:].rearrange("p (r c) -> p r c", c=WP)   # [128, 132, 132]
    vv = xp4v[:].rearrange("p (r c) -> p r c", c=WP)

    ar_in = dram.tile([1, 8], F32)
    ar_out = dram.tile([1, 8], F32)
    nc.gpsimd.dma_start(ar_in[:], sums[0:1, :])
    nc.gpsimd.collective_compute(
        "AllReduce", ALU.add,
        replica_groups=[list(range(N_CORES))],
        ins=[ar_in.opt()], outs=[ar_out.opt()],
    )
    gs = main.tile([1, 8], F32)
    nc.gpsimd.dma_start(gs[:], ar_out[:])

    early.release()
    psum_hv = tc.alloc_tile_pool(name="psum_hv", bufs=1, space="PSUM")

    # ---------------- directional convs -> F planes (bf16) ----------------
    # fhv 0:64 = F_H [64, 129, 128]: top(0:32) rr=F_top[max(rr-2,0)];
    #   bot(32:64) rr=F_bot[min(rr+4,131)], psum col w+1.
    #   select reads: P = [:, h+1, w], Q = [:, h, w]
    # fhv 64:128 = F_V [64, 128, 129]: left(0:32) cc=F_left[r+1,max(cc-2,0)];
    #   right(32:64) cc=F_right[r, min(cc+4,131)]
    #   select reads: P = [:, h, w+1], Q = [:, h, w]
    fhv = late.tile([128, 129 * 128], BF16)
    f_h = fhv[0:64].rearrange("p (r c) -> p r c", r=129)
    f_v = fhv[64:128].rearrange("p (r c) -> p r c", r=128)

    _ev_state = [0]
    def _evict(dst, srcv):
        e = _ev_state[0] % 2
        _ev_state[0] += 1
        if e == 0:
            nc.scalar.activation(dst, srcv, ACT.Identity)
        else:
            nc.vector.tensor_copy(dst, srcv)

    # interleave H and V 6-row chunks; psum [64, 2, 512] (bank-aligned halves)
    for k in range(22):
        r0 = 6 * k
        ph = psum_hv.tile([64, 2, 512], F32, tag="ph")
        for u in range(2):
            nc.tensor.matmul(ph[:, u, 0:387], w_h[:],
                             vh[:, r0 + 3 * u:r0 + 3 * u + 3, 0:129],
                             start=True, stop=True)
        phv = ph[:, :, 0:387].rearrange("p a (r c) -> p a r c", c=129)
        # top: rr = r+2, r in [0..126]
        lo, hi = r0, min(r0 + 6, 127)
        if lo < hi:
            for hf, a, b, oo in _psum_pieces(lo - r0, hi - r0):
                _evict(f_h[0:32, lo + oo + 2:lo + oo + 2 + (b - a), :],
                       phv[0:32, hf, a:b, 0:128])
        # bot: rr = r-4, r in [4..131]
        lo, hi = max(r0, 4), min(r0 + 6, 132)
        if lo < hi:
            for hf, a, b, oo in _psum_pieces(lo - r0, hi - r0):
                _evict(f_h[32:64, lo + oo - 4:lo + oo - 4 + (b - a), :],
                       phv[32:64, hf, a:b, 1:129])

        pvt = psum_hv.tile([64, 2, 512], F32, tag="pv")
        hp0 = 6 * k
        nrows = min(6, 129 - hp0)
        if nrows <= 0:
            continue
        for u in range(2):
            nr = min(3, 129 - hp0 - 3 * u)
            if nr > 0:
                nc.tensor.matmul(pvt[:, u, 0:nr * 132], w_v[:],
                                 vv[:, hp0 + 3 * u:hp0 + 3 * u + nr, 0:132],
                                 start=True, stop=True)
        pvv = pvt[:, :, 0:396].rearrange("p a (r c) -> p a r c", c=132)
        # left (0:32): r = hp-1, hp in [1..128]; cc = col+2 for cols 0..126
        lo, hi = max(hp0, 1), min(hp0 + nrows, 129)
        if lo < hi:
            for hf, a, b, oo in _psum_pieces(lo - hp0, hi - hp0):
                _evict(f_v[0:32, lo + oo - 1:lo + oo - 1 + (b - a), 2:129],
                       pvv[0:32, hf, a:b, 0:127])
        # right (32:64): r = hp in [0..127]; cc <- psum col cc+4
        lo, hi = hp0, min(hp0 + nrows, 128)
        if lo < hi:
            for hf, a, b, oo in _psum_pieces(lo - hp0, hi - hp0):
                _evict(f_v[32:64, lo + oo:lo + oo + (b - a), 0:128],
                       pvv[32:64, hf, a:b, 4:132])

    # one-time dup rows/cols (self copies after main fills)
    nc.vector.tensor_copy(f_h[0:32, 0, :], f_h[0:32, 2, :])
    nc.vector.tensor_copy(f_h[0:32, 1, :], f_h[0:32, 2, :])
    nc.vector.tensor_copy(f_h[32:64, 128, :], f_h[32:64, 127, :])
    nc.vector.tensor_copy(f_v[0:32, :, 0:1], f_v[0:32, :, 2:3])
    nc.vector.tensor_copy(f_v[0:32, :, 1:2], f_v[0:32, :, 2:3])
    nc.vector.tensor_copy(f_v[32:64, :, 128:129], f_v[32:64, :, 127:128])

    psum_hv.release()
    psum_bc = tc.alloc_tile_pool(name="psum_bc", bufs=2, space="PSUM")
    xp4vp.release()

    # ---------------- rb corner strip matmuls ----------------
    g11rhs = main.tile([32, 132], BF16)
    nc.sync.dma_start(g11rhs[:], vh[0:32, :, 131])
    pg = psum_acc.tile([32, 512], F32, tag="acc")
    nc.tensor.matmul(pg[0:32, 0:132], w_g11[:], g11rhs[:],
                     start=True, stop=True)
    g11e = main.tile([32, 134], F32)
    nc.scalar.activation(g11e[:, 0:132], pg[0:32, 0:132], ACT.Identity)
    nc.scalar.activation(g11e[:, 132:133], pg[0:32, 131:132], ACT.Identity)
    nc.scalar.activation(g11e[:, 133:134], pg[0:32, 131:132], ACT.Identity)
    pg2 = psum_acc.tile([32, 512], F32, tag="acc")
    nc.tensor.matmul(pg2[0:32, 0:132], w_g15[:], vh[0:32, 131, 0:132],
                     start=True, stop=True)
    g15e = main.tile([32, 133], F32)
    nc.scalar.activation(g15e[:, 1:133], pg2[0:32, 0:132], ACT.Identity)
    nc.scalar.activation(g15e[:, 0:1], pg2[0:32, 0:1], ACT.Identity)
    xp4hp.release()

    # ---------------- BN consts from the early AllReduce ----------------
    # tile_wait_until: the AllReduce lands late — keep these ops from being
    # scheduled ahead of the conv-phase evictions in the engine queues
    with tc.tile_wait_until(0.21):
        mu = main.tile([1, 4], F32)
        nc.vector.tensor_scalar(mu[:], gs[:, 0:4], INV_N, None, ALU.mult)
        ex2 = main.tile([1, 4], F32)
        nc.vector.tensor_scalar(ex2[:], gs[:, 4:8], INV_N, None, ALU.mult)
        var = main.tile([1, 4], F32)
        nc.vector.tensor_tensor(var[:], mu[:], mu[:], ALU.mult)
        nc.vector.tensor_tensor(var[:], ex2[:], var[:], ALU.subtract)
        nc.vector.tensor_scalar(var[:], var[:], EPS, None, ALU.add)
        sd = main.tile([1, 4], F32)
        nc.scalar.sqrt(sd[:], var[:])
        rsq = main.tile([1, 4], F32)
        nc.vector.reciprocal(rsq[:], sd[:])
        zscale = main.tile([1, 4], F32)
        nc.vector.tensor_tensor(zscale[:], rsq[:], bnp[:, 0:4], ALU.mult)
        zbias = main.tile([1, 4], F32)
        nc.vector.tensor_tensor(zbias[:], mu[:], zscale[:], ALU.mult)
        nc.vector.tensor_tensor(zbias[:], bnp[:, 4:8], zbias[:], ALU.subtract)
        cons = main.tile([128, 8], F32)   # bcast: zscale 0:4, zbias 4:8
        nc.gpsimd.partition_broadcast(cons[:, 0:4], zscale[:])
        nc.gpsimd.partition_broadcast(cons[:, 4:8], zbias[:])

    # ------------- coefficient planes, dir-stacked [128, 4, 128] ----------
    # dir stack order: (top, left, bot, right); minus dirs = 0:2, plus = 2:4
    coeffp = tc.alloc_tile_pool(name="coeffp", bufs=1)
    _hint = tc.tile_wait_until(0.215)
    _hint.__enter__()
    ih = main.tile([H, W], mybir.dt.int32)
    nc.gpsimd.iota(ih[:], pattern=[[0, W]], base=0, channel_multiplier=1)
    iw = main.tile([H, W], mybir.dt.int32)
    nc.gpsimd.iota(iw[:], pattern=[[1, W]], base=0, channel_multiplier=0)
    idxb1 = coeffp.tile([128, 4, W], F32)   # IDX - 1 (minus) / IDX + 4 (plus)
    idxb2 = coeffp.tile([128, 4, W], F32)   # IDX (minus) / IDX + 4 (plus)
    for col, (srci, o1, o2) in enumerate([(0, -1.0, 0.0), (1, -1.0, 0.0),
                                          (0, 4.0, 4.0), (1, 4.0, 4.0)]):
        srct = ih if srci == 0 else iw
        nc.vector.tensor_scalar(idxb1[:, col, :], srct[:], o1, None, ALU.add)
        nc.vector.tensor_scalar(idxb2[:, col, :], srct[:], o2, None, ALU.add)

    z = coeffp.tile([128, 4, W], F32)
    for ch in range(4):
        nc.vector.tensor_scalar(z[:, ch, :], preS[:, ch, :],
                                cons[:, ch:ch + 1], cons[:, 4 + ch:5 + ch],
                                ALU.mult, ALU.add)
    gb = coeffp.tile([128, 4, W], F32)
    nc.scalar.activation(gb[:].rearrange("p a b -> p (a b)"),
                         z[:].rearrange("p a b -> p (a b)"), ACT.Sigmoid)
    nc.vector.tensor_scalar(gb[:], gb[:], 2.0, None, ALU.mult)
    m2 = coeffp.tile([128, 4, W], F32)
    nc.vector.tensor_scalar(m2[:], z[:], 0.0, None, ALU.is_gt)

    q = coeffp.tile([128, 4, W], F32)
    nc.vector.tensor_tensor(q[:, 0:2, :], idxb1[:, 0:2, :], m2[:, 0:2, :],
                            ALU.subtract)
    nc.vector.tensor_tensor(q[:, 2:4, :], idxb1[:, 2:4, :], m2[:, 2:4, :],
                            ALU.add)
    nc.vector.tensor_scalar(q[:, 0:2, :], q[:, 0:2, :], 0.0, None, ALU.max)
    nc.vector.tensor_scalar(q[:, 2:4, :], q[:, 2:4, :], 131.0, None, ALU.min)
    pcl = coeffp.tile([128, 4, W], F32)
    nc.vector.tensor_tensor(pcl[:, 0:2, :], idxb2[:, 0:2, :], gb[:, 0:2, :],
                            ALU.subtract)
    nc.vector.tensor_tensor(pcl[:, 2:4, :], idxb2[:, 2:4, :], gb[:, 2:4, :],
                            ALU.add)
    nc.vector.tensor_scalar(pcl[:, 0:2, :], pcl[:, 0:2, :], 0.0, None, ALU.max)
    nc.vector.tensor_scalar(pcl[:, 2:4, :], pcl[:, 2:4, :], 131.0, None,
                            ALU.min)
    wlt = coeffp.tile([128, 4, W], F32)
    nc.vector.tensor_tensor(wlt[:], q[:], pcl[:], ALU.subtract)
    nc.vector.tensor_scalar(wlt[:], wlt[:], 1.0, None, ALU.add)
    # cq = wlt * [m2, m2, 1-m2, 1-m2]; cp = wlt - cq
    bmul = coeffp.tile([128, 4, W], F32)
    nc.vector.tensor_tensor(bmul[:], wlt[:], m2[:], ALU.mult)
    cqS = coeffp.tile([128, 4, W], BF16)
    nc.vector.tensor_copy(cqS[:, 0:2, :], bmul[:, 0:2, :])
    nc.vector.tensor_tensor(cqS[:, 2:4, :], wlt[:, 2:4, :], bmul[:, 2:4, :],
                            ALU.subtract)
    cpS = coeffp.tile([128, 4, W], BF16)
    nc.vector.tensor_tensor(cpS[:], wlt[:], cqS[:], ALU.subtract)

    # ab_dram rows (order = S partition groups): cp: [top, bot, left, right]
    # cq rows 4:8 same order. Stack cols (t,l,b,r) -> rows via 2 DMAs each.
    ab_dram = dram.tile([8, HW], BF16)
    for base, tsrc in ((0, cpS), (4, cqS)):
        nc.sync.dma_start(
            ab_dram[base:base + 2, :].rearrange("d (h w) -> h d w", w=W),
            tsrc[:, 0:4:2, :])          # cols (top, bot) -> rows +0,+1
        nc.sync.dma_start(
            ab_dram[base + 2:base + 4, :].rearrange("d (h w) -> h d w", w=W),
            tsrc[:, 1:4:2, :])          # cols (left, right) -> rows +2,+3


    # ---------------- rb corner strip coefficients ----------------
    # bottom strip at (h, 127): dir col 2; [128,1] partition-major
    ihf1 = main.tile([128, 1], F32)
    nc.vector.tensor_copy(ihf1[:], ih[:, 0:1])
    m2c = main.tile([128, 1], F32)
    nc.vector.tensor_copy(m2c[:], m2[:, 2, 127:128])
    gbc = main.tile([128, 1], F32)
    nc.vector.tensor_copy(gbc[:], gb[:, 2, 127:128])
    qs = main.tile([128, 1], F32)
    nc.vector.tensor_tensor(qs[:], ihf1[:], m2c[:], ALU.add)
    nc.vector.tensor_scalar(qs[:], qs[:], 5.0, 131.0, ALU.add, ALU.min)
    ps_ = main.tile([128, 1], F32)
    nc.vector.tensor_tensor(ps_[:], ihf1[:], gbc[:], ALU.add)
    nc.vector.tensor_scalar(ps_[:], ps_[:], 4.0, 131.0, ALU.add, ALU.min)
    wrb = main.tile([128, 1], F32)
    nc.vector.tensor_tensor(wrb[:], qs[:], ps_[:], ALU.subtract)
    nc.vector.tensor_scalar(wrb[:], wrb[:], 1.0, None, ALU.subtract)
    cbs = main.tile([128, 1], F32)
    nc.vector.tensor_tensor(cbs[:], wrb[:], m2c[:], ALU.mult)
    cas = main.tile([128, 1], F32)
    nc.vector.tensor_tensor(cas[:], wrb[:], cbs[:], ALU.subtract)
    strip_dram = dram.tile([4, 128], F32)
    nc.sync.dma_start(strip_dram[0:1, :], cas[:])
    nc.sync.dma_start(strip_dram[1:2, :], cbs[:])

    # left strip at (127, w): dir col 1; [1, 128] via DMA off partition 127
    m2r = main.tile([1, 128], F32)
    nc.sync.dma_start(m2r[:], m2[127:128, 1, :])
    gbr = main.tile([1, 128], F32)
    nc.sync.dma_start(gbr[:], gb[127:128, 1, :])
    iwf1 = main.tile([1, 128], F32)
    nc.vector.tensor_copy(iwf1[:], iw[0:1, :])
    qs2 = main.tile([1, 128], F32)
    nc.vector.tensor_tensor(qs2[:], iwf1[:], m2r[:], ALU.subtract)
    nc.vector.tensor_scalar(qs2[:], qs2[:], 0.0, None, ALU.max)
    ps2 = main.tile([1, 128], F32)
    nc.vector.tensor_tensor(ps2[:], iwf1[:], gbr[:], ALU.subtract)
    nc.vector.tensor_scalar(ps2[:], ps2[:], 0.0, None, ALU.max)
    wrb2 = main.tile([1, 128], F32)
    nc.vector.tensor_tensor(wrb2[:], qs2[:], ps2[:], ALU.subtract)
    nc.vector.tensor_scalar(wrb2[:], wrb2[:], 1.0, None, ALU.subtract)
    cb2 = main.tile([1, 128], F32)
    nc.vector.tensor_tensor(cb2[:], wrb2[:], m2r[:], ALU.mult)
    ca2 = main.tile([1, 128], F32)
    nc.vector.tensor_tensor(ca2[:], wrb2[:], cb2[:], ALU.subtract)
    nc.sync.dma_start(strip_dram[2:3, :], ca2[:])
    nc.sync.dma_start(strip_dram[3:4, :], cb2[:])

    strips = main.tile([32, 4, 128], F32)
    nc.sync.dma_start(strips[:],
                      strip_dram[:].unsqueeze(0).broadcast_to([32, 4, 128]))
    t1 = main.tile([32, 128], F32)
    nc.vector.tensor_tensor(t1[:], strips[:, 0, :], g11e[:, 5:133], ALU.mult)
    t2 = main.tile([32, 128], F32)
    nc.vector.tensor_tensor(t2[:], strips[:, 1, :], g11e[:, 6:134], ALU.mult)
    nc.vector.tensor_tensor(t1[:], t1[:], t2[:], ALU.add)
    t3 = main.tile([32, 128], F32)
    nc.vector.tensor_tensor(t3[:], strips[:, 2, :], g15e[:, 1:129], ALU.mult)
    t4 = main.tile([32, 128], F32)
    nc.vector.tensor_tensor(t4[:], strips[:, 3, :], g15e[:, 0:128], ALU.mult)
    nc.vector.tensor_tensor(t3[:], t3[:], t4[:], ALU.add)
    _hint.__exit__(None, None, None)

    # ---------------- select stage + final matmuls ----------------
    # Software-pipelined: stage A(k) (bc matmuls -> bf16 evict -> 2x DVE
    # multiplies) is emitted before stage B(k-1) (acc matmuls + out) so the
    # Tensor stream always has ready work.
    coeffp.release()
    chunkL2 = tc.alloc_tile_pool(name="chunkL2", bufs=3)
    out_flat = out_d[:].rearrange("o h w -> o (h w)")

    def sel_stageA(kc):
        r0 = kc * SEL_ROWS
        pix0 = r0 * W
        npix = SEL_ROWS * W                      # 2048
        s1 = chunkL2.tile([128, SEL_ROWS, W], BF16, tag="s1")   # P-stack
        s2 = chunkL2.tile([128, SEL_ROWS, W], BF16, tag="s2")   # Q-stack
        ab_cp = chunkL2.tile([4, npix], BF16, tag="ab_cp")
        nc.gpsimd.dma_start(ab_cp[:], ab_dram[0:4, pix0:pix0 + npix])
        ab_cq = chunkL2.tile([4, npix], BF16, tag="ab_cq")
        nc.gpsimd.dma_start(ab_cq[:], ab_dram[4:8, pix0:pix0 + npix])
        cen = chunkL2.tile([OUT, npix], F32, tag="cen")
        nc.gpsimd.dma_start(cen[:], y_dram[28:60, pix0:pix0 + npix])
        for hf in range(2):
            rr = r0 + 8 * hf            # image rows rr..rr+8 in this half
            rel = 8 * hf
            cpb = chunkL2.tile([128, 8, W], BF16, tag="cpb")
            cqb = chunkL2.tile([128, 8, W], BF16, tag="cqb")
            for u in range(2):
                o_l = 1024 * hf + 512 * u
                pbc = psum_bc.tile([128, 2, 512], F32, tag="pbc")
                nc.tensor.matmul(pbc[:, 0, :], b4[:],
                                 ab_cp[:, o_l:o_l + 512],
                                 start=True, stop=True)
                nc.tensor.matmul(pbc[:, 1, :], b4[:],
                                 ab_cq[:, o_l:o_l + 512],
                                 start=True, stop=True)
                # evict to bf16 SBUF so the multiplies hit DVE 2x mode
                nc.scalar.activation(
                    cpb[:, 4 * u:4 * u + 4, :].rearrange("p r w -> p (r w)"),
                    pbc[:, 0, :], ACT.Identity)
                nc.scalar.activation(
                    cqb[:, 4 * u:4 * u + 4, :].rearrange("p r w -> p (r w)"),
                    pbc[:, 1, :], ACT.Identity)
            nc.vector.tensor_tensor(s1[0:64, rel:rel + 8, :], cpb[0:64],
                                    f_h[:, rr + 1:rr + 9, :], ALU.mult)
            nc.vector.tensor_tensor(s2[0:64, rel:rel + 8, :], cqb[0:64],
                                    f_h[:, rr:rr + 8, :], ALU.mult)
            nc.vector.tensor_tensor(s1[64:128, rel:rel + 8, :], cpb[64:128],
                                    f_v[:, rr:rr + 8, 1:129], ALU.mult)
            nc.vector.tensor_tensor(s2[64:128, rel:rel + 8, :], cqb[64:128],
                                    f_v[:, rr:rr + 8, 0:128], ALU.mult)
        return s1, s2, cen

    def sel_stageB(kc, s1, s2, cen):
        r0 = kc * SEL_ROWS
        pix0 = r0 * W
        npix = SEL_ROWS * W
        out_c = chunkL2.tile([OUT, SEL_ROWS, W], F32, tag="out_c")
        s1v = s1[:].rearrange("p r w -> p (r w)")
        s2v = s2[:].rearrange("p r w -> p (r w)")
        for i in range(npix // 512):
            acc = psum_acc.tile([32, 512], F32, tag="acc")
            nc.tensor.matmul(acc[:], w_s[:], s1v[:, 512 * i:512 * (i + 1)],
                             start=True, stop=False)
            nc.tensor.matmul(acc[:], w_s[:], s2v[:, 512 * i:512 * (i + 1)],
                             start=False, stop=True)
            nc.vector.tensor_tensor(
                out_c[:, 4 * i:4 * i + 4, :].rearrange("p r w -> p (r w)"),
                acc[:], cen[:, 512 * i:512 * (i + 1)], ALU.add)
        nc.vector.tensor_tensor(out_c[:, :, 127], out_c[:, :, 127],
                                t1[:, r0:r0 + SEL_ROWS], ALU.add)
        if kc == NCHUNK_SEL - 1:
            nc.vector.tensor_tensor(out_c[:, SEL_ROWS - 1, :],
                                    out_c[:, SEL_ROWS - 1, :], t3[:], ALU.add)
        nc.sync.dma_start(out_flat[:, pix0:pix0 + npix],
                          out_c[:].rearrange("o r w -> o (r w)"))

    prev = sel_stageA(0)
    for kc in range(1, NCHUNK_SEL):
        cur = sel_stageA(kc)
        sel_stageB(kc - 1, *prev)
        prev = cur
    sel_stageB(NCHUNK_SEL - 1, *prev)

    chunkL2.release()
    late.release()
    psum_bc.release()
    psum_acc.release()
    dram.release()
    main.release()


def _psum_pieces(rlo, rhi):
    """Split psum row range [rlo, rhi) in [0,6) into per-half pieces.

    Returns list of (half, half_rlo, half_rhi, out_row_offset_from_rlo)."""
    assert 0 <= rlo < rhi <= 6
    pieces = []
    if rlo < 3:
        e = min(rhi, 3)
        pieces.append((0, rlo, e, 0))
    if rhi > 3:
        s = max(rlo, 3)
        pieces.append((1, s - 3, rhi - 3, s - rlo))
    return pieces


_CACHED = {}


def _get_nc():
    if 'nc' not in _CACHED:
        nc = bacc.Bacc(None, target_bir_lowering=False)
        build(nc)
        nc.compile()
        _CACHED['nc'] = nc
    return _CACHED['nc']


def kernel(**inputs):
    nc = _get_nc()
    wd = host_prep(inputs)
    x = np.asarray(inputs['x'], np.float32)
    in_maps = []
    for i in range(N_CORES):
        m = {'x': np.ascontiguousarray(x[i])}
        m.update(wd)
        in_maps.append(m)
    res = run_bass_kernel_spmd(nc, in_maps, core_ids=list(range(N_CORES)))
    outs = [res.results[i]['out'] for i in range(N_CORES)]
    return np.stack(outs, axis=0)


if __name__ == '__main__':
    nc = _get_nc()
    print("build+compile OK")

